# revision 18
# baseline (speedup 1.0000x reference)
"""Trainium2 Bass kernel for nn_MultiHeadedAttention_9706626089976.

Multi-scale windowed attention over video frames + 3x3 output conv.

v2: 3 SPMD launches on 8 NeuronCores (host does sharding/permutes only):
  A : 1x1-conv QKV projections, data-parallel over the 16 frames (2/core),
      bf16 in/out, N=1024 moving tiles.
  B : all three attention scales in ONE launch; per core = (sample, query
      quarter) for each scale; bf16 transport halves the HBM traffic that
      bounds the small scales.
  C : 3x3 conv + bias + LeakyReLU(0.2), data-parallel over frames (2/core);
      bf16 weights resident in SBUF (loaded once), fused Lrelu activation.

Attention computes scores TRANSPOSED (scoresT[key, q] = K^T-chunks @ Q) so
softmax needs no max-pass/no transposes and exp(scoresT) is directly the
lhsT operand of the P@V matmul.
"""

import hashlib
import math
import os
import shutil

import ml_dtypes
import numpy as np

import concourse.bass as bass
import concourse.bass2jax as bass2jax
import concourse.mybir as mybir
import concourse.tile as tile
from concourse import bacc
from concourse.bass_utils import run_bass_kernel_spmd

BF16NP = ml_dtypes.bfloat16

# Deterministic on-disk NEFF cache keyed on BIR content (walrus compile of
# a launch is minutes; identical BIR always yields the same NEFF).
_NEFF_CACHE_DIR = "/tmp/neff_cache"
_orig_compile_bir_kernel = bass2jax.compile_bir_kernel


def _cached_compile_bir_kernel(bir_json, tmpdir, neff_name="file.neff"):
    data = bir_json if isinstance(bir_json, bytes) else bir_json.encode()
    h = hashlib.sha256(data).hexdigest()
    cpath = os.path.join(_NEFF_CACHE_DIR, h + ".neff")
    if os.path.exists(cpath):
        dst = os.path.join(tmpdir, neff_name)
        shutil.copyfile(cpath, dst)
        return dst
    path = _orig_compile_bir_kernel(bir_json, tmpdir, neff_name=neff_name)
    try:
        os.makedirs(_NEFF_CACHE_DIR, exist_ok=True)
        tmp = cpath + ".tmp." + str(os.getpid())
        shutil.copyfile(path, tmp)
        os.replace(tmp, cpath)
    except OSError:
        pass
    return path


bass2jax.compile_bir_kernel = _cached_compile_bir_kernel

# Problem constants (hardcoded per harness contract).
BT, B, T, C, H, W = 16, 2, 8, 768, 64, 64
DK = 256
FRAMES_PER_CORE = BT // 8
PATCHSIZE = [(16, 16), (8, 8), (4, 4)]
N_CORES = 8

F32 = mybir.dt.float32
BF16 = mybir.dt.bfloat16

# (n, d) per scale; nq = n // 4 (4-way query split per sample).
SCALES = []
for _si, (_pw, _ph) in enumerate(PATCHSIZE):
    _oh, _ow = H // _ph, W // _pw
    SCALES.append((T * _oh * _ow, DK * _ph * _pw))

_BUILD_CACHE = {}

# test.py sets TRACE=True to collect per-launch HW exec times into TIMES.
TRACE = False
TIMES = []


def _run(nc, in_maps, cores, label):
    res = run_bass_kernel_spmd(nc, in_maps, core_ids=cores, trace=TRACE)
    if TRACE:
        TIMES.append((label, res.exec_time_ns))
    return res


def _bacc():
    return bacc.Bacc("TRN2", target_bir_lowering=False, debug=False,
                     num_devices=N_CORES)


# ---------------------------------------------------------------- launch A
def _build_proj():
    """Per core: x2 [2,768,4096] bf16 -> qkv [3,2,768,4096] bf16."""
    nc = _bacc()
    x_in = nc.dram_tensor("x2", [FRAMES_PER_CORE, C, H * W], BF16,
                          kind="ExternalInput").ap()
    w_in = nc.dram_tensor("wT", [C, 3 * C], BF16, kind="ExternalInput").ap()
    b_in = nc.dram_tensor("bqkv", [3, C], F32, kind="ExternalInput").ap()
    out = nc.dram_tensor("qkv", [3, FRAMES_PER_CORE, C, H * W], BF16,
                         kind="ExternalOutput").ap()
    CC = C // 128  # 6 channel chunks
    NB = 512       # moving-dim block (PSUM bank limit: 512 f32)
    n_pb = (H * W) // NB
    with tile.TileContext(nc) as tc:
        with tc.tile_pool(name="wp", bufs=1) as wp, \
             tc.tile_pool(name="xp", bufs=2) as xp, \
             tc.tile_pool(name="op", bufs=4) as op, \
             tc.tile_pool(name="pp", bufs=3, space="PSUM") as pp:
            # chunked loads: first matmul starts after 1/3 of w + 1/8 of x
            # instead of the full 10 MB (region-level deps)
            w_t = wp.tile([128, CC, 3 * C], BF16)
            w_r = w_in.rearrange("(c k) n -> k c n", k=128)
            for p in range(3):
                nc.sync.dma_start(out=w_t[:, :, p * C:(p + 1) * C],
                                  in_=w_r[:, :, p * C:(p + 1) * C])
            bias_t = wp.tile([128, 3, CC], F32)
            nc.sync.dma_start(out=bias_t,
                              in_=b_in.rearrange("p (c k) -> k p c", k=128))
            for f in range(FRAMES_PER_CORE):
                x_t = xp.tile([128, CC, H * W], BF16)
                x_r = x_in[f].rearrange("(c k) p -> k c p", k=128)
                for pb in range(n_pb):
                    nc.sync.dma_start(
                        out=x_t[:, :, pb * NB:(pb + 1) * NB],
                        in_=x_r[:, :, pb * NB:(pb + 1) * NB])
                for p in range(3):
                    for oc in range(CC):
                        for pb in range(n_pb):
                            ps = pp.tile([128, NB], F32)
                            for ic in range(CC):
                                nc.tensor.matmul(
                                    ps,
                                    w_t[:, ic, p * C + oc * 128:p * C + oc * 128 + 128],
                                    x_t[:, ic, pb * NB:(pb + 1) * NB],
                                    start=(ic == 0), stop=(ic == CC - 1))
                            ot = op.tile([128, NB], BF16)
                            nc.scalar.activation(
                                out=ot, in_=ps,
                                func=mybir.ActivationFunctionType.Identity,
                                bias=bias_t[:, p, oc:oc + 1], scale=1.0)
                            nc.sync.dma_start(
                                out=out[p, f, oc * 128:(oc + 1) * 128,
                                        pb * NB:(pb + 1) * NB],
                                in_=ot)
    nc.compile()
    return nc


# ---------------------------------------------------------------- launch B
def _emit_attn_scale(nc, pools, q_in, k_in, v_in, y_out, n, d, nq, d_pv, kbw):
    """Emit one scale's windowed attention. Per core:
      Q packed [128, n_dc*nq] bf16, K packed [n_kbp, n_g, 128, DCG*kbw]
      bf16 (host pre-tiled so every DMA is fully contiguous),
      V [n, d_pv] bf16 -> y [nq, d_pv] bf16.
    scoresT[key, q] accumulated in PSUM over d; exp on ACT (scale folded);
    key-sums via ones-matmul; P@V with expT as lhsT; normalization folded
    into the PSUM->SBUF copy of y. d_pv < d means this core only computes
    a column-slice of y (scale-0: full queries, quarter of V columns).
    Pools are shared across scales (padded tiles, fixed tags) so buffer
    rotation serializes memory reuse."""
    qp, kp, ep, vp, yp, sp, pp, py, pq = pools
    scale = 1.0 / math.sqrt(d)
    n_kb = n // 128           # key blocks
    n_dc = d // 128           # contraction chunks
    DCG = 32                  # d-chunks per streamed K group (4096 rows)
    n_g = n_dc // DCG
    n_sub = kbw // 128        # key blocks per packed K load
    n_qb = max(1, nq // 128)  # query blocks (nq may be < 128)
    dj_cols = 512             # V column block
    n_dj = d_pv // dj_cols

    v_r = v_in.rearrange("(c k) e -> k c e", k=128)

    # Q resident when it fits; else (scale 0: full queries x full d) stream
    # it group-wise like K — safe since each chunk is read once (n_kb == 1).
    stream_q = n_dc * nq > 16384
    if stream_q:
        assert n_kb == n_sub == 1
        q_v = None
    else:
        q_t = qp.tile([128, 16384], BF16, tag="q")
        q_v = q_t[:, :n_dc * nq].rearrange("k (c n) -> k c n", n=nq)
        nc.sync.dma_start(out=q_v,
                          in_=q_in.rearrange("k (c n) -> k c n", n=nq))
    ones_t = sp.tile([128, 2], BF16, tag="one")
    nc.vector.memset(ones_t, 1.0)
    exp_t = ep.tile([128, 8192], BF16, tag="e")
    exp_v = exp_t[:, :n_kb * nq].rearrange("k (b n) -> k b n", n=nq)

    for kbp in range(n_kb // n_sub):
        st_list = [pp.tile([128, 512], F32, tag="s", name=f"st{sub}")
                   for sub in range(n_sub)]
        for g in range(n_g):
            k_t = kp.tile([128, DCG * kbw], BF16, tag="k")
            nc.sync.dma_start(out=k_t, in_=k_in[kbp, g])
            k_v = k_t.rearrange("k (c n) -> k c n", c=DCG)
            if stream_q:
                q_g = qp.tile([128, DCG * nq], BF16, tag="qg")
                nc.sync.dma_start(
                    out=q_g,
                    in_=q_in[:, g * DCG * nq:(g + 1) * DCG * nq])
                q_gv = q_g.rearrange("k (c n) -> k c n", n=nq)
            for sub in range(n_sub):
                for c_ in range(DCG):
                    dc = g * DCG + c_
                    nc.tensor.matmul(
                        st_list[sub][:, :nq],
                        k_v[:, c_, sub * 128:(sub + 1) * 128],
                        q_gv[:, c_, :] if stream_q else q_v[:, dc, :],
                        start=(dc == 0), stop=(dc == n_dc - 1))
        for sub in range(n_sub):
            kb = kbp * n_sub + sub
            nc.scalar.activation(out=exp_v[:, kb, :],
                                 in_=st_list[sub][:, :nq],
                                 func=mybir.ActivationFunctionType.Exp,
                                 scale=scale)
    # per-query key-sums, partition-oriented: sums[q] over keys.
    sums_ps = pq.tile([128, 8], F32, tag="sm")
    for qb in range(n_qb):
        mq = min(128, nq - qb * 128)
        for kb in range(n_kb):
            nc.tensor.matmul(
                sums_ps[:mq, 2 * qb:2 * qb + 2],
                exp_v[:, kb, qb * 128:qb * 128 + mq],
                ones_t[:, 0:2],
                start=(kb == 0), stop=(kb == n_kb - 1))
    mq0 = min(128, nq)
    rq_t = sp.tile([128, 4], F32, tag="r")
    nc.vector.reciprocal(
        out=rq_t[:mq0, :n_qb],
        in_=sums_ps.rearrange("k (b two) -> k b two", two=2)[:mq0, :n_qb, 0])

    for dj in range(n_dj):
        v_t = vp.tile([128, 8192], BF16, tag="v")
        v_v = v_t.rearrange("k (b e) -> k b e", e=dj_cols)
        nc.sync.dma_start(out=v_v[:, :n_kb, :],
                          in_=v_r[:, :, dj * dj_cols:(dj + 1) * dj_cols])
        for qb in range(n_qb):
            mq = min(128, nq - qb * 128)
            y_ps = py.tile([128, dj_cols], F32, tag="y")
            for kb in range(n_kb):
                nc.tensor.matmul(
                    y_ps[:mq, :],
                    exp_v[:, kb, qb * 128:qb * 128 + mq],
                    v_v[:, kb, :],
                    start=(kb == 0), stop=(kb == n_kb - 1))
            y_t = yp.tile([128, dj_cols], BF16, tag="o")
            nc.vector.tensor_scalar_mul(
                y_t[:mq, :], y_ps[:mq, :], rq_t[:mq, qb:qb + 1])
            nc.sync.dma_start(
                out=y_out[qb * 128:qb * 128 + mq,
                          dj * dj_cols:(dj + 1) * dj_cols],
                in_=y_t[:mq, :])


def _attn_params(si):
    """(n, d, nq, d_pv, kbw) for scale si. Scale 0: full queries per core,
    V-column quarter (its n=128 makes query-splitting dispatch-bound);
    scales 1/2: query quarter, full V columns. kbw = keys per packed K
    load (256 gives 512B+ contiguous DMA runs)."""
    n, d = SCALES[si]
    if si == 0:
        return n, d, n, d // 4, 128
    return n, d, n // 4, d, 256


def _build_attn():
    """One launch, all 3 scales. Per core = (sample, query quarter)."""
    nc = _bacc()
    ins, outs = [], []
    for si in range(3):
        n, d, nq, d_pv, kbw = _attn_params(si)
        n_dc = d // 128
        n_g = n_dc // 32
        n_kbp = n // kbw
        ins.append((
            nc.dram_tensor(f"q{si}", [128, n_dc * nq], BF16,
                           kind="ExternalInput").ap(),
            nc.dram_tensor(f"k{si}", [n_kbp, n_g, 128, 32 * kbw], BF16,
                           kind="ExternalInput").ap(),
            nc.dram_tensor(f"v{si}", [n, d_pv], BF16,
                           kind="ExternalInput").ap(),
        ))
        outs.append(
            nc.dram_tensor(f"y{si}", [nq, d_pv], BF16,
                           kind="ExternalOutput").ap())
    with tile.TileContext(nc) as tc:
        with tc.tile_pool(name="qp", bufs=2) as qp, \
             tc.tile_pool(name="kp", bufs=3) as kp, \
             tc.tile_pool(name="ep", bufs=2) as ep, \
             tc.tile_pool(name="vp", bufs=2) as vp, \
             tc.tile_pool(name="yp", bufs=4) as yp, \
             tc.tile_pool(name="sp", bufs=2) as sp, \
             tc.tile_pool(name="pp", bufs=3, space="PSUM") as pp, \
             tc.tile_pool(name="py", bufs=3, space="PSUM") as py, \
             tc.tile_pool(name="pq", bufs=2, space="PSUM") as pq:
            pools = (qp, kp, ep, vp, yp, sp, pp, py, pq)
            # big scale first: its long QK phase overlaps later scales' DMA
            for si in (2, 1, 0):
                n, d, nq, d_pv, kbw = _attn_params(si)
                q_in, k_in, v_in = ins[si]
                _emit_attn_scale(nc, pools, q_in, k_in, v_in, outs[si],
                                 n, d, nq, d_pv, kbw)
    nc.compile()
    return nc


# ---------------------------------------------------------------- launch C
def _build_conv():
    """Per core: y2pad [2,768,66,66] bf16, woT [9,768,768] bf16, bo [768]
    -> out [2,768,4096] f32 with bias + LeakyReLU(0.2)."""
    nc = _bacc()
    x_in = nc.dram_tensor("y2pad", [FRAMES_PER_CORE, C, 66 * 66], BF16,
                          kind="ExternalInput").ap()
    w_in = nc.dram_tensor("woT", [9, C, C], BF16, kind="ExternalInput").ap()
    b_in = nc.dram_tensor("bo", [C], F32, kind="ExternalInput").ap()
    out = nc.dram_tensor("out", [FRAMES_PER_CORE, C, H * W], F32,
                         kind="ExternalOutput").ap()
    CC = C // 128
    NR = 8  # output rows per block (N = NR*64 = 512, PSUM bank limit)
    n_rb = H // NR
    with tile.TileContext(nc) as tc:
        with tc.tile_pool(name="wp", bufs=1) as wp, \
             tc.tile_pool(name="xp", bufs=2) as xp, \
             tc.tile_pool(name="op", bufs=3) as op, \
             tc.tile_pool(name="pp", bufs=3, space="PSUM") as pp:
            # all weights resident: [128(ic%128), 9, CC(ic//128), 768(oc)]
            w_t = wp.tile([128, 9, CC, C], BF16)
            nc.sync.dma_start(
                out=w_t, in_=w_in.rearrange("s (c k) o -> k s c o", k=128))
            bias_t = wp.tile([128, CC], F32)
            nc.sync.dma_start(out=bias_t,
                              in_=b_in.rearrange("(c k) -> k c", k=128))
            for f in range(FRAMES_PER_CORE):
                x_t = xp.tile([128, CC, 66 * 66], BF16)
                nc.sync.dma_start(
                    out=x_t, in_=x_in[f].rearrange("(c k) p -> k c p", k=128))
                x_v = x_t.rearrange("k c (r q) -> k c r q", r=66)
                for oc in range(CC):
                    for rb in range(n_rb):
                        ps = pp.tile([128, NR * 64], F32)
                        first = True
                        for dy in range(3):
                            for dx in range(3):
                                for ic in range(CC):
                                    y0 = rb * NR + dy
                                    rhs = x_v[:, ic, y0:y0 + NR, dx:dx + 64]
                                    nc.tensor.matmul(
                                        ps,
                                        w_t[:, dy * 3 + dx, ic,
                                            oc * 128:(oc + 1) * 128],
                                        rhs,
                                        start=first,
                                        stop=(dy == 2 and dx == 2 and ic == CC - 1))
                                    first = False
                        zt = op.tile([128, NR * 64], F32, tag="zt")
                        nc.scalar.activation(
                            out=zt, in_=ps,
                            func=mybir.ActivationFunctionType.Identity,
                            bias=bias_t[:, oc:oc + 1], scale=1.0)
                        lt = op.tile([128, NR * 64], F32, tag="lt")
                        nc.vector.tensor_scalar_mul(lt, zt, 0.2)
                        ot = op.tile([128, NR * 64], F32, tag="ot")
                        nc.vector.tensor_tensor(
                            out=ot, in0=zt, in1=lt, op=mybir.AluOpType.max)
                        nc.sync.dma_start(
                            out=out[f, oc * 128:(oc + 1) * 128,
                                    rb * (NR * 64):(rb + 1) * (NR * 64)],
                            in_=ot)
    nc.compile()
    return nc


def _build_conv_wino():
    """1D (width) Winograd F(2,3) conv: 1.5x fewer MACs than direct.
    Per core: y2pad [2,768,66,66] bf16, wWx [12,768,768] bf16 (px*3+dy,
    ic, oc = G-transformed weights), bo [768] f32 -> out [2,768,4096] f32.

    Per 16-output-row batch: T1 = B^T-combine of input cols (4 px slices,
    DVE); per (px, oc-chunk): PSUM accumulates sum_dy sum_ic W~[px,dy]^T @
    T1[rows+dy]; DVE A^T-combines the 4 px results into even/odd output
    columns; ACT applies bias + LeakyReLU."""
    nc = _bacc()
    x_in = nc.dram_tensor("y2pad", [FRAMES_PER_CORE, C, 66 * 66], BF16,
                          kind="ExternalInput").ap()
    w_in = nc.dram_tensor("wWx", [12, C, C], BF16, kind="ExternalInput").ap()
    b_in = nc.dram_tensor("bo", [C], F32, kind="ExternalInput").ap()
    out = nc.dram_tensor("out", [FRAMES_PER_CORE, C, H * W], F32,
                         kind="ExternalOutput").ap()
    CC = C // 128
    ADD, SUB = mybir.AluOpType.add, mybir.AluOpType.subtract
    with tile.TileContext(nc) as tc:
        with tc.tile_pool(name="wp", bufs=1) as wp, \
             tc.tile_pool(name="xp", bufs=2) as xp, \
             tc.tile_pool(name="tp", bufs=2) as tp, \
             tc.tile_pool(name="ap", bufs=2) as acp, \
             tc.tile_pool(name="op", bufs=2) as op, \
             tc.tile_pool(name="bp", bufs=1) as bp, \
             tc.tile_pool(name="pp", bufs=3, space="PSUM") as pp:
            bias_t = bp.tile([128, CC], F32)
            nc.sync.dma_start(out=bias_t,
                              in_=b_in.rearrange("(c k) -> k c", k=128))
            for oh_ in range(2):  # oc halves (3 chunks each)
                w_t = wp.tile([128, 12, CC, 384], BF16, tag="w")
                nc.sync.dma_start(
                    out=w_t,
                    in_=w_in[:, :, oh_ * 384:(oh_ + 1) * 384].rearrange(
                        "s (c k) o -> k s c o", k=128))
                for f in range(FRAMES_PER_CORE):
                    for tb in range(4):  # 16-output-row batches
                        y0 = tb * 16
                        x_t = xp.tile([128, CC, 18 * 66], BF16, tag="x")
                        nc.sync.dma_start(
                            out=x_t,
                            in_=x_in[f][:, y0 * 66:(y0 + 18) * 66].rearrange(
                                "(c k) p -> k c p", k=128))
                        # xe[..., t, 0] = col 2t, xe[..., t, 1] = col 2t+1
                        xe = x_t.rearrange("k c (r t two) -> k c r t two",
                                           two=2, t=33)
                        t1 = tp.tile([128, CC, 18, 128], BF16, tag="t1")
                        t1v = t1.rearrange("k c r (p t) -> k c r p t", p=4)
                        # u0=d0-d2, u1=d1+d2, u2=d2-d1, u3=d1-d3
                        nc.vector.tensor_tensor(
                            out=t1v[:, :, :, 0, :], op=SUB,
                            in0=xe[:, :, :, 0:32, 0], in1=xe[:, :, :, 1:33, 0])
                        nc.vector.tensor_tensor(
                            out=t1v[:, :, :, 1, :], op=ADD,
                            in0=xe[:, :, :, 0:32, 1], in1=xe[:, :, :, 1:33, 0])
                        nc.vector.tensor_tensor(
                            out=t1v[:, :, :, 2, :], op=SUB,
                            in0=xe[:, :, :, 1:33, 0], in1=xe[:, :, :, 0:32, 1])
                        nc.vector.tensor_tensor(
                            out=t1v[:, :, :, 3, :], op=SUB,
                            in0=xe[:, :, :, 0:32, 1], in1=xe[:, :, :, 1:33, 1])
                        acc = acp.tile([128, 3, 16, 64], F32, tag="acc")
                        accv = acc.rearrange("k c r (t two) -> k c r t two",
                                             two=2)
                        for px in range(4):
                            for occ in range(3):
                                psz = pp.tile([128, 512], F32, tag="z")
                                first = True
                                for dy in range(3):
                                    for ic in range(CC):
                                        nc.tensor.matmul(
                                            psz,
                                            w_t[:, px * 3 + dy, ic,
                                                occ * 128:(occ + 1) * 128],
                                            t1v[:, ic, dy:dy + 16, px, :],
                                            start=first,
                                            stop=(dy == 2 and ic == CC - 1))
                                        first = False
                                zv = psz.rearrange("k (r t) -> k r t", r=16)
                                ev = accv[:, occ, :, :, 0]
                                od = accv[:, occ, :, :, 1]
                                # A^T: even = z0+z1+z2 ; odd = z1-z2-z3
                                if px == 0:
                                    nc.vector.tensor_copy(out=ev, in_=zv)
                                elif px == 1:
                                    nc.vector.tensor_tensor(
                                        out=ev, op=ADD, in0=ev, in1=zv)
                                    nc.vector.tensor_copy(out=od, in_=zv)
                                elif px == 2:
                                    nc.vector.tensor_tensor(
                                        out=ev, op=ADD, in0=ev, in1=zv)
                                    nc.vector.tensor_tensor(
                                        out=od, op=SUB, in0=od, in1=zv)
                                else:
                                    nc.vector.tensor_tensor(
                                        out=od, op=SUB, in0=od, in1=zv)
                        for occ in range(3):
                            zt = op.tile([128, 16 * 64], F32, tag="zt")
                            nc.scalar.activation(
                                out=zt, in_=acc[:, occ],
                                func=mybir.ActivationFunctionType.Identity,
                                bias=bias_t[:, oh_ * 3 + occ:oh_ * 3 + occ + 1],
                                scale=1.0)
                            lt = op.tile([128, 16 * 64], F32, tag="lt")
                            nc.vector.tensor_scalar_mul(lt, zt, 0.2)
                            ot = op.tile([128, 16 * 64], F32, tag="ot")
                            nc.vector.tensor_tensor(
                                out=ot, in0=zt, in1=lt,
                                op=mybir.AluOpType.max)
                            nc.sync.dma_start(
                                out=out[f, (oh_ * 3 + occ) * 128:
                                        (oh_ * 3 + occ + 1) * 128,
                                        tb * 1024:(tb + 1) * 1024],
                                in_=ot)
    nc.compile()
    return nc


# winograd weight transform (host, weight preprocessing)
_GX = np.array([[1.0, 0.0, 0.0],
                [0.5, 0.5, 0.5],
                [0.5, -0.5, 0.5],
                [0.0, 0.0, 1.0]], dtype=np.float32)

CONV_IMPL = "wino"  # "wino" | "direct"


# ------------------------------------------------------------------- host
def _pack_q(qsd):
    """[nq, d] bf16 -> [128, n_dc*nq] contiguous partition-major tiles."""
    nq, d = qsd.shape
    n_dc = d // 128
    return np.ascontiguousarray(
        qsd.T.reshape(n_dc, 128, nq).transpose(1, 0, 2)).reshape(
            128, n_dc * nq)


def _pack_k(ksd, kbw):
    """[n, d] bf16 -> [n_kbp, n_g, 128, 32*kbw] contiguous K tiles."""
    n, d = ksd.shape
    n_g = d // (32 * 128)
    n_kbp = n // kbw
    kt = ksd.T.reshape(n_g, 32, 128, n_kbp, kbw)
    return np.ascontiguousarray(kt.transpose(3, 0, 2, 1, 4)).reshape(
        n_kbp, n_g, 128, 32 * kbw)


def _windows(z, si, ph, pw):
    """z [bt, c, h, w] -> [b, n, D] for scale si."""
    oh, ow = H // ph, W // pw
    zz = z[:, si * DK:(si + 1) * DK].reshape(B, T, DK, oh, ph, ow, pw)
    zz = zz.transpose(0, 1, 3, 5, 2, 4, 6)
    return np.ascontiguousarray(zz.reshape(B, T * oh * ow, DK * ph * pw))


def _unwindows(y, si, ph, pw):
    """y [b, n, D] -> [bt, DK, h, w] for scale si."""
    oh, ow = H // ph, W // pw
    yy = y.reshape(B, T, oh, ow, DK, ph, pw).transpose(0, 1, 4, 2, 5, 3, 6)
    return yy.reshape(BT, DK, H, W)


def _get(name, builder, *args):
    key = (name,) + args
    if key not in _BUILD_CACHE:
        _BUILD_CACHE[key] = builder(*args)
    return _BUILD_CACHE[key]


def kernel(x, m, wq, bq, wk, bk, wv, bv, wo, bo, b, c):
    x = np.asarray(x, dtype=np.float32)
    assert x.shape == (BT, C, H, W) and int(b) == B and int(c) == C
    cores = list(range(N_CORES))

    # ---- launch A: QKV projections, 2 frames/core
    wT = np.ascontiguousarray(np.concatenate(
        [np.asarray(w)[:, :, 0, 0].T for w in (wq, wk, wv)], axis=1,
        dtype=np.float32)).astype(BF16NP)
    bqkv = np.stack([np.asarray(bq), np.asarray(bk), np.asarray(bv)]
                    ).astype(np.float32)
    x_flat = x.reshape(BT, C, H * W).astype(BF16NP)
    nc_a = _get("proj", _build_proj)
    in_maps = [{"x2": np.ascontiguousarray(
                    x_flat[i * FRAMES_PER_CORE:(i + 1) * FRAMES_PER_CORE]),
                "wT": wT, "bqkv": bqkv} for i in cores]
    res = _run(nc_a, in_maps, cores, "proj")
    qkv = np.concatenate([np.asarray(r["qkv"]) for r in res.results], axis=1)
    q_all = qkv[0].reshape(BT, C, H, W)
    k_all = qkv[1].reshape(BT, C, H, W)
    v_all = qkv[2].reshape(BT, C, H, W)

    # ---- launch B: all scales; scales 1/2 split queries, scale 0 splits
    # V columns (per sample, 4 cores each)
    nc_b = _get("attn", _build_attn)
    in_maps = [dict() for _ in cores]
    for si, (pw_, ph_) in enumerate(PATCHSIZE):
        n, d, nq, d_pv, kbw = _attn_params(si)
        qw = _windows(q_all, si, ph_, pw_)   # [b, n, D] bf16
        kw = _windows(k_all, si, ph_, pw_)
        vw = _windows(v_all, si, ph_, pw_)
        kpk = [_pack_k(kw[s], kbw) for s in range(B)]
        qpk = [_pack_q(qw[s]) for s in range(B)] if si == 0 else None
        for i in cores:
            s, qq = i // 4, i % 4
            if si == 0:
                in_maps[i][f"q{si}"] = qpk[s]
                in_maps[i][f"v{si}"] = np.ascontiguousarray(
                    vw[s][:, qq * d_pv:(qq + 1) * d_pv])
            else:
                in_maps[i][f"q{si}"] = _pack_q(qw[s, qq * nq:(qq + 1) * nq])
                in_maps[i][f"v{si}"] = vw[s]
            in_maps[i][f"k{si}"] = kpk[s]
    res = _run(nc_b, in_maps, cores, "attn")
    y_scales = []
    for si, (pw_, ph_) in enumerate(PATCHSIZE):
        n, d, nq, d_pv, kbw = _attn_params(si)
        y = np.empty((B, n, d), dtype=BF16NP)
        for i in cores:
            s, qq = i // 4, i % 4
            yi = np.asarray(res.results[i][f"y{si}"])
            if si == 0:
                y[s, :, qq * d_pv:(qq + 1) * d_pv] = yi
            else:
                y[s, qq * nq:(qq + 1) * nq] = yi
        y_scales.append(_unwindows(y, si, ph_, pw_))

    y_cat = np.concatenate(y_scales, axis=1)  # [bt, C, h, w] bf16

    # ---- launch C: 3x3 conv + bias + LeakyReLU, 2 frames/core
    y_pad = np.zeros((BT, C, 66, 66), dtype=BF16NP)
    y_pad[:, :, 1:65, 1:65] = y_cat
    y_pad = y_pad.reshape(BT, C, 66 * 66)
    bo_ = np.asarray(bo, dtype=np.float32)
    if CONV_IMPL == "wino":
        # wWx[px*3+dy][i, o] = sum_kx Gx[px,kx] wo[o,i,dy,kx]
        wWx = np.einsum('pk,oidk->pdio', _GX,
                        np.asarray(wo, dtype=np.float32),
                        optimize=True).reshape(12, C, C).astype(BF16NP)
        nc_c = _get("convw", _build_conv_wino)
        in_maps = [{"y2pad": np.ascontiguousarray(
                        y_pad[i * FRAMES_PER_CORE:(i + 1) * FRAMES_PER_CORE]),
                    "wWx": np.ascontiguousarray(wWx), "bo": bo_}
                   for i in cores]
    else:
        woT = np.ascontiguousarray(
            np.asarray(wo, dtype=np.float32).transpose(2, 3, 1, 0)
            .reshape(9, C, C)).astype(BF16NP)
        nc_c = _get("conv", _build_conv)
        in_maps = [{"y2pad": np.ascontiguousarray(
                        y_pad[i * FRAMES_PER_CORE:(i + 1) * FRAMES_PER_CORE]),
                    "woT": woT, "bo": bo_} for i in cores]
    res = _run(nc_c, in_maps, cores, "conv")
    out = np.concatenate([np.asarray(r["out"], dtype=np.float32)
                          for r in res.results], axis=0)
    return out.reshape(BT, C, H, W)


# revision 20
# speedup vs baseline: 1.0900x; 1.0900x over previous
"""Trainium2 Bass kernel for nn_MultiHeadedAttention_9706626089976.

Multi-scale windowed attention over video frames + 3x3 output conv.

v2: 3 SPMD launches on 8 NeuronCores (host does sharding/permutes only):
  A : 1x1-conv QKV projections, data-parallel over the 16 frames (2/core),
      bf16 in/out, N=1024 moving tiles.
  B : all three attention scales in ONE launch; per core = (sample, query
      quarter) for each scale; bf16 transport halves the HBM traffic that
      bounds the small scales.
  C : 3x3 conv + bias + LeakyReLU(0.2), data-parallel over frames (2/core);
      bf16 weights resident in SBUF (loaded once), fused Lrelu activation.

Attention computes scores TRANSPOSED (scoresT[key, q] = K^T-chunks @ Q) so
softmax needs no max-pass/no transposes and exp(scoresT) is directly the
lhsT operand of the P@V matmul.
"""

import hashlib
import math
import os
import shutil

import ml_dtypes
import numpy as np

import concourse.bass as bass
import concourse.bass2jax as bass2jax
import concourse.mybir as mybir
import concourse.tile as tile
from concourse import bacc
from concourse.bass_utils import run_bass_kernel_spmd

BF16NP = ml_dtypes.bfloat16

# Deterministic on-disk NEFF cache keyed on BIR content (walrus compile of
# a launch is minutes; identical BIR always yields the same NEFF).
_NEFF_CACHE_DIR = "/tmp/neff_cache"
_orig_compile_bir_kernel = bass2jax.compile_bir_kernel


def _cached_compile_bir_kernel(bir_json, tmpdir, neff_name="file.neff"):
    data = bir_json if isinstance(bir_json, bytes) else bir_json.encode()
    h = hashlib.sha256(data).hexdigest()
    cpath = os.path.join(_NEFF_CACHE_DIR, h + ".neff")
    if os.path.exists(cpath):
        dst = os.path.join(tmpdir, neff_name)
        shutil.copyfile(cpath, dst)
        return dst
    path = _orig_compile_bir_kernel(bir_json, tmpdir, neff_name=neff_name)
    try:
        os.makedirs(_NEFF_CACHE_DIR, exist_ok=True)
        tmp = cpath + ".tmp." + str(os.getpid())
        shutil.copyfile(path, tmp)
        os.replace(tmp, cpath)
    except OSError:
        pass
    return path


bass2jax.compile_bir_kernel = _cached_compile_bir_kernel

# Problem constants (hardcoded per harness contract).
BT, B, T, C, H, W = 16, 2, 8, 768, 64, 64
DK = 256
FRAMES_PER_CORE = BT // 8
PATCHSIZE = [(16, 16), (8, 8), (4, 4)]
N_CORES = 8

F32 = mybir.dt.float32
BF16 = mybir.dt.bfloat16

# (n, d) per scale; nq = n // 4 (4-way query split per sample).
SCALES = []
for _si, (_pw, _ph) in enumerate(PATCHSIZE):
    _oh, _ow = H // _ph, W // _pw
    SCALES.append((T * _oh * _ow, DK * _ph * _pw))

_BUILD_CACHE = {}

# test.py sets TRACE=True to collect per-launch HW exec times into TIMES.
TRACE = False
TIMES = []


def _run(nc, in_maps, cores, label):
    res = run_bass_kernel_spmd(nc, in_maps, core_ids=cores, trace=TRACE)
    if TRACE:
        TIMES.append((label, res.exec_time_ns))
    return res


def _bacc():
    return bacc.Bacc("TRN2", target_bir_lowering=False, debug=False,
                     num_devices=N_CORES)


# ---------------------------------------------------------------- launch A
def _build_proj():
    """Per core: x2 [2,768,4096] bf16 -> qkv [3,2,768,4096] bf16."""
    nc = _bacc()
    x_in = nc.dram_tensor("x2", [FRAMES_PER_CORE, C, H * W], BF16,
                          kind="ExternalInput").ap()
    w_in = nc.dram_tensor("wT", [C, 3 * C], BF16, kind="ExternalInput").ap()
    b_in = nc.dram_tensor("bqkv", [3, C], F32, kind="ExternalInput").ap()
    out = nc.dram_tensor("qkv", [3, FRAMES_PER_CORE, C, H * W], BF16,
                         kind="ExternalOutput").ap()
    CC = C // 128  # 6 channel chunks
    NB = 512       # moving-dim block (PSUM bank limit: 512 f32)
    n_pb = (H * W) // NB
    with tile.TileContext(nc) as tc:
        with tc.tile_pool(name="wp", bufs=1) as wp, \
             tc.tile_pool(name="xp", bufs=2) as xp, \
             tc.tile_pool(name="op", bufs=4) as op, \
             tc.tile_pool(name="pp", bufs=3, space="PSUM") as pp:
            w_t = wp.tile([128, CC, 3 * C], BF16)
            nc.sync.dma_start(out=w_t, in_=w_in.rearrange("(c k) n -> k c n", k=128))
            bias_t = wp.tile([128, 3, CC], F32)
            nc.sync.dma_start(out=bias_t,
                              in_=b_in.rearrange("p (c k) -> k p c", k=128))
            for f in range(FRAMES_PER_CORE):
                x_t = xp.tile([128, CC, H * W], BF16)
                nc.sync.dma_start(
                    out=x_t, in_=x_in[f].rearrange("(c k) p -> k c p", k=128))
                for p in range(3):
                    for oc in range(CC):
                        for pb in range(n_pb):
                            ps = pp.tile([128, NB], F32)
                            for ic in range(CC):
                                nc.tensor.matmul(
                                    ps,
                                    w_t[:, ic, p * C + oc * 128:p * C + oc * 128 + 128],
                                    x_t[:, ic, pb * NB:(pb + 1) * NB],
                                    start=(ic == 0), stop=(ic == CC - 1))
                            ot = op.tile([128, NB], BF16)
                            nc.scalar.activation(
                                out=ot, in_=ps,
                                func=mybir.ActivationFunctionType.Identity,
                                bias=bias_t[:, p, oc:oc + 1], scale=1.0)
                            nc.sync.dma_start(
                                out=out[p, f, oc * 128:(oc + 1) * 128,
                                        pb * NB:(pb + 1) * NB],
                                in_=ot)
    nc.compile()
    return nc


# ---------------------------------------------------------------- launch B
def _emit_attn_scale(nc, pools, q_in, k_in, v_in, y_out, n, d, nq, d_pv, kbw):
    """Emit one scale's windowed attention. Per core:
      Q packed [128, n_dc*nq] bf16, K packed [n_kbp, n_g, 128, DCG*kbw]
      bf16 (host pre-tiled so every DMA is fully contiguous),
      V [n, d_pv] bf16 -> y [nq, d_pv] bf16.
    scoresT[key, q] accumulated in PSUM over d; exp on ACT (scale folded);
    key-sums via ones-matmul; P@V with expT as lhsT; normalization folded
    into the PSUM->SBUF copy of y. d_pv < d means this core only computes
    a column-slice of y (scale-0: full queries, quarter of V columns).
    Pools are shared across scales (padded tiles, fixed tags) so buffer
    rotation serializes memory reuse."""
    qp, kp, ep, vp, yp, sp, pp, py, pq = pools
    scale = 1.0 / math.sqrt(d)
    n_kb = n // 128           # key blocks
    n_dc = d // 128           # contraction chunks
    DCG = 32                  # d-chunks per streamed K group (4096 rows)
    n_g = n_dc // DCG
    n_sub = kbw // 128        # key blocks per packed K load
    n_qb = max(1, nq // 128)  # query blocks (nq may be < 128)
    dj_cols = 512             # V column block
    n_dj = d_pv // dj_cols

    v_r = v_in.rearrange("(c k) e -> k c e", k=128)

    # Q resident when it fits; else (scale 0: full queries x full d) stream
    # it group-wise like K — safe since each chunk is read once (n_kb == 1).
    stream_q = n_dc * nq > 16384
    if stream_q:
        assert n_kb == n_sub == 1
        q_v = None
    else:
        q_t = qp.tile([128, 16384], BF16, tag="q")
        q_v = q_t[:, :n_dc * nq].rearrange("k (c n) -> k c n", n=nq)
        nc.sync.dma_start(out=q_v,
                          in_=q_in.rearrange("k (c n) -> k c n", n=nq))
    ones_t = sp.tile([128, 2], BF16, tag="one")
    nc.vector.memset(ones_t, 1.0)
    exp_t = ep.tile([128, 8192], BF16, tag="e")
    exp_v = exp_t[:, :n_kb * nq].rearrange("k (b n) -> k b n", n=nq)

    for kbp in range(n_kb // n_sub):
        st_list = [pp.tile([128, 512], F32, tag="s", name=f"st{sub}")
                   for sub in range(n_sub)]
        for g in range(n_g):
            k_t = kp.tile([128, DCG * kbw], BF16, tag="k")
            nc.sync.dma_start(out=k_t, in_=k_in[kbp, g])
            k_v = k_t.rearrange("k (c n) -> k c n", c=DCG)
            if stream_q:
                q_g = qp.tile([128, DCG * nq], BF16, tag="qg")
                nc.sync.dma_start(
                    out=q_g,
                    in_=q_in[:, g * DCG * nq:(g + 1) * DCG * nq])
                q_gv = q_g.rearrange("k (c n) -> k c n", n=nq)
            for sub in range(n_sub):
                for c_ in range(DCG):
                    dc = g * DCG + c_
                    nc.tensor.matmul(
                        st_list[sub][:, :nq],
                        k_v[:, c_, sub * 128:(sub + 1) * 128],
                        q_gv[:, c_, :] if stream_q else q_v[:, dc, :],
                        start=(dc == 0), stop=(dc == n_dc - 1))
        for sub in range(n_sub):
            kb = kbp * n_sub + sub
            nc.scalar.activation(out=exp_v[:, kb, :],
                                 in_=st_list[sub][:, :nq],
                                 func=mybir.ActivationFunctionType.Exp,
                                 scale=scale)
    # per-query key-sums, partition-oriented: sums[q] over keys.
    sums_ps = pq.tile([128, 8], F32, tag="sm")
    for qb in range(n_qb):
        mq = min(128, nq - qb * 128)
        for kb in range(n_kb):
            nc.tensor.matmul(
                sums_ps[:mq, 2 * qb:2 * qb + 2],
                exp_v[:, kb, qb * 128:qb * 128 + mq],
                ones_t[:, 0:2],
                start=(kb == 0), stop=(kb == n_kb - 1))
    mq0 = min(128, nq)
    rq_t = sp.tile([128, 4], F32, tag="r")
    nc.vector.reciprocal(
        out=rq_t[:mq0, :n_qb],
        in_=sums_ps.rearrange("k (b two) -> k b two", two=2)[:mq0, :n_qb, 0])

    for dj in range(n_dj):
        v_t = vp.tile([128, 8192], BF16, tag="v")
        v_v = v_t.rearrange("k (b e) -> k b e", e=dj_cols)
        nc.sync.dma_start(out=v_v[:, :n_kb, :],
                          in_=v_r[:, :, dj * dj_cols:(dj + 1) * dj_cols])
        for qb in range(n_qb):
            mq = min(128, nq - qb * 128)
            y_ps = py.tile([128, dj_cols], F32, tag="y")
            for kb in range(n_kb):
                nc.tensor.matmul(
                    y_ps[:mq, :],
                    exp_v[:, kb, qb * 128:qb * 128 + mq],
                    v_v[:, kb, :],
                    start=(kb == 0), stop=(kb == n_kb - 1))
            y_t = yp.tile([128, dj_cols], BF16, tag="o")
            nc.vector.tensor_scalar_mul(
                y_t[:mq, :], y_ps[:mq, :], rq_t[:mq, qb:qb + 1])
            nc.sync.dma_start(
                out=y_out[qb * 128:qb * 128 + mq,
                          dj * dj_cols:(dj + 1) * dj_cols],
                in_=y_t[:mq, :])


def _attn_params(si):
    """(n, d, nq, d_pv, kbw) for scale si. Scale 0: full queries per core,
    V-column quarter (its n=128 makes query-splitting dispatch-bound);
    scales 1/2: query quarter, full V columns. kbw = keys per packed K
    load (256 gives 512B+ contiguous DMA runs)."""
    n, d = SCALES[si]
    if si == 0:
        return n, d, n, d // 4, 128
    return n, d, n // 4, d, 256


def _build_attn():
    """One launch, all 3 scales. Per core = (sample, query quarter)."""
    nc = _bacc()
    ins, outs = [], []
    for si in range(3):
        n, d, nq, d_pv, kbw = _attn_params(si)
        n_dc = d // 128
        n_g = n_dc // 32
        n_kbp = n // kbw
        ins.append((
            nc.dram_tensor(f"q{si}", [128, n_dc * nq], BF16,
                           kind="ExternalInput").ap(),
            nc.dram_tensor(f"k{si}", [n_kbp, n_g, 128, 32 * kbw], BF16,
                           kind="ExternalInput").ap(),
            nc.dram_tensor(f"v{si}", [n, d_pv], BF16,
                           kind="ExternalInput").ap(),
        ))
        outs.append(
            nc.dram_tensor(f"y{si}", [nq, d_pv], BF16,
                           kind="ExternalOutput").ap())
    with tile.TileContext(nc) as tc:
        with tc.tile_pool(name="qp", bufs=2) as qp, \
             tc.tile_pool(name="kp", bufs=3) as kp, \
             tc.tile_pool(name="ep", bufs=2) as ep, \
             tc.tile_pool(name="vp", bufs=2) as vp, \
             tc.tile_pool(name="yp", bufs=4) as yp, \
             tc.tile_pool(name="sp", bufs=2) as sp, \
             tc.tile_pool(name="pp", bufs=3, space="PSUM") as pp, \
             tc.tile_pool(name="py", bufs=3, space="PSUM") as py, \
             tc.tile_pool(name="pq", bufs=2, space="PSUM") as pq:
            pools = (qp, kp, ep, vp, yp, sp, pp, py, pq)
            # big scale first: its long QK phase overlaps later scales' DMA
            for si in (2, 1, 0):
                n, d, nq, d_pv, kbw = _attn_params(si)
                q_in, k_in, v_in = ins[si]
                _emit_attn_scale(nc, pools, q_in, k_in, v_in, outs[si],
                                 n, d, nq, d_pv, kbw)
    nc.compile()
    return nc


# ---------------------------------------------------------------- launch C
def _build_conv():
    """Per core: y2pad [2,768,66,66] bf16, woT [9,768,768] bf16, bo [768]
    -> out [2,768,4096] f32 with bias + LeakyReLU(0.2)."""
    nc = _bacc()
    x_in = nc.dram_tensor("y2pad", [FRAMES_PER_CORE, C, 66 * 66], BF16,
                          kind="ExternalInput").ap()
    w_in = nc.dram_tensor("woT", [9, C, C], BF16, kind="ExternalInput").ap()
    b_in = nc.dram_tensor("bo", [C], F32, kind="ExternalInput").ap()
    out = nc.dram_tensor("out", [FRAMES_PER_CORE, C, H * W], F32,
                         kind="ExternalOutput").ap()
    CC = C // 128
    NR = 8  # output rows per block (N = NR*64 = 512, PSUM bank limit)
    n_rb = H // NR
    with tile.TileContext(nc) as tc:
        with tc.tile_pool(name="wp", bufs=1) as wp, \
             tc.tile_pool(name="xp", bufs=2) as xp, \
             tc.tile_pool(name="op", bufs=3) as op, \
             tc.tile_pool(name="pp", bufs=3, space="PSUM") as pp:
            # all weights resident: [128(ic%128), 9, CC(ic//128), 768(oc)]
            w_t = wp.tile([128, 9, CC, C], BF16)
            nc.sync.dma_start(
                out=w_t, in_=w_in.rearrange("s (c k) o -> k s c o", k=128))
            bias_t = wp.tile([128, CC], F32)
            nc.sync.dma_start(out=bias_t,
                              in_=b_in.rearrange("(c k) -> k c", k=128))
            for f in range(FRAMES_PER_CORE):
                x_t = xp.tile([128, CC, 66 * 66], BF16)
                nc.sync.dma_start(
                    out=x_t, in_=x_in[f].rearrange("(c k) p -> k c p", k=128))
                x_v = x_t.rearrange("k c (r q) -> k c r q", r=66)
                for oc in range(CC):
                    for rb in range(n_rb):
                        ps = pp.tile([128, NR * 64], F32)
                        first = True
                        for dy in range(3):
                            for dx in range(3):
                                for ic in range(CC):
                                    y0 = rb * NR + dy
                                    rhs = x_v[:, ic, y0:y0 + NR, dx:dx + 64]
                                    nc.tensor.matmul(
                                        ps,
                                        w_t[:, dy * 3 + dx, ic,
                                            oc * 128:(oc + 1) * 128],
                                        rhs,
                                        start=first,
                                        stop=(dy == 2 and dx == 2 and ic == CC - 1))
                                    first = False
                        zt = op.tile([128, NR * 64], F32, tag="zt")
                        nc.scalar.activation(
                            out=zt, in_=ps,
                            func=mybir.ActivationFunctionType.Identity,
                            bias=bias_t[:, oc:oc + 1], scale=1.0)
                        lt = op.tile([128, NR * 64], F32, tag="lt")
                        nc.vector.tensor_scalar_mul(lt, zt, 0.2)
                        ot = op.tile([128, NR * 64], F32, tag="ot")
                        nc.vector.tensor_tensor(
                            out=ot, in0=zt, in1=lt, op=mybir.AluOpType.max)
                        nc.sync.dma_start(
                            out=out[f, oc * 128:(oc + 1) * 128,
                                    rb * (NR * 64):(rb + 1) * (NR * 64)],
                            in_=ot)
    nc.compile()
    return nc


def _build_conv_wino():
    """1D (width) Winograd F(2,3) conv: 1.5x fewer MACs than direct.
    Per core: y2pad [2,768,66,66] bf16, wWx [12,768,768] bf16 (px*3+dy,
    ic, oc = G-transformed weights), bo [768] f32 -> out [2,768,4096] f32.

    Per 16-output-row batch: T1 = B^T-combine of input cols (4 px slices,
    DVE); per (px, oc-chunk): PSUM accumulates sum_dy sum_ic W~[px,dy]^T @
    T1[rows+dy]; DVE A^T-combines the 4 px results into even/odd output
    columns; ACT applies bias + LeakyReLU."""
    nc = _bacc()
    x_in = nc.dram_tensor("y2pad", [FRAMES_PER_CORE, C, 66 * 66], BF16,
                          kind="ExternalInput").ap()
    w_in = nc.dram_tensor("wWx", [12, C, C], BF16, kind="ExternalInput").ap()
    b_in = nc.dram_tensor("bo", [C], F32, kind="ExternalInput").ap()
    out = nc.dram_tensor("out", [FRAMES_PER_CORE, C, H * W], F32,
                         kind="ExternalOutput").ap()
    CC = C // 128
    ADD, SUB = mybir.AluOpType.add, mybir.AluOpType.subtract
    with tile.TileContext(nc) as tc:
        with tc.tile_pool(name="wp", bufs=1) as wp, \
             tc.tile_pool(name="xp", bufs=2) as xp, \
             tc.tile_pool(name="tp", bufs=1) as tp, \
             tc.tile_pool(name="ap", bufs=2) as acp, \
             tc.tile_pool(name="op", bufs=2) as op, \
             tc.tile_pool(name="bp", bufs=1) as bp, \
             tc.tile_pool(name="pp", bufs=3, space="PSUM") as pp:
            bias_t = bp.tile([128, CC], F32)
            nc.sync.dma_start(out=bias_t,
                              in_=b_in.rearrange("(c k) -> k c", k=128))
            for oh_ in range(2):  # oc halves (3 chunks each)
                w_t = wp.tile([128, 12, CC, 384], BF16, tag="w")
                nc.sync.dma_start(
                    out=w_t,
                    in_=w_in[:, :, oh_ * 384:(oh_ + 1) * 384].rearrange(
                        "s (c k) o -> k s c o", k=128))
                for f in range(FRAMES_PER_CORE):
                    for tb in range(4):  # 16-output-row batches
                        y0 = tb * 16
                        x_t = xp.tile([128, CC, 18 * 66], BF16, tag="x")
                        nc.sync.dma_start(
                            out=x_t,
                            in_=x_in[f][:, y0 * 66:(y0 + 18) * 66].rearrange(
                                "(c k) p -> k c p", k=128))
                        # xe[..., t, 0] = col 2t, xe[..., t, 1] = col 2t+1
                        xe = x_t.rearrange("k c (r t two) -> k c r t two",
                                           two=2, t=33)
                        t1 = tp.tile([128, CC, 18, 128], BF16, tag="t1")
                        t1v = t1.rearrange("k c r (p t) -> k c r p t", p=4)
                        # u0=d0-d2, u1=d1+d2, u2=d2-d1, u3=d1-d3
                        nc.vector.tensor_tensor(
                            out=t1v[:, :, :, 0, :], op=SUB,
                            in0=xe[:, :, :, 0:32, 0], in1=xe[:, :, :, 1:33, 0])
                        nc.vector.tensor_tensor(
                            out=t1v[:, :, :, 1, :], op=ADD,
                            in0=xe[:, :, :, 0:32, 1], in1=xe[:, :, :, 1:33, 0])
                        nc.vector.tensor_tensor(
                            out=t1v[:, :, :, 2, :], op=SUB,
                            in0=xe[:, :, :, 1:33, 0], in1=xe[:, :, :, 0:32, 1])
                        nc.vector.tensor_tensor(
                            out=t1v[:, :, :, 3, :], op=SUB,
                            in0=xe[:, :, :, 0:32, 1], in1=xe[:, :, :, 1:33, 1])
                        acc = acp.tile([128, 3, 16, 64], F32, tag="acc")
                        accv = acc.rearrange("k c r (t two) -> k c r t two",
                                             two=2)
                        for px in range(4):
                            for occ in range(3):
                                psz = pp.tile([128, 512], F32, tag="z")
                                first = True
                                for dy in range(3):
                                    for ic in range(CC):
                                        nc.tensor.matmul(
                                            psz,
                                            w_t[:, px * 3 + dy, ic,
                                                occ * 128:(occ + 1) * 128],
                                            t1v[:, ic, dy:dy + 16, px, :],
                                            start=first,
                                            stop=(dy == 2 and ic == CC - 1))
                                        first = False
                                zv = psz.rearrange("k (r t) -> k r t", r=16)
                                ev = accv[:, occ, :, :, 0]
                                od = accv[:, occ, :, :, 1]
                                # A^T: even = z0+z1+z2 ; odd = z1-z2-z3
                                if px == 0:
                                    nc.vector.tensor_copy(out=ev, in_=zv)
                                elif px == 1:
                                    nc.vector.tensor_tensor(
                                        out=ev, op=ADD, in0=ev, in1=zv)
                                    nc.vector.tensor_copy(out=od, in_=zv)
                                elif px == 2:
                                    nc.vector.tensor_tensor(
                                        out=ev, op=ADD, in0=ev, in1=zv)
                                    nc.vector.tensor_tensor(
                                        out=od, op=SUB, in0=od, in1=zv)
                                else:
                                    nc.vector.tensor_tensor(
                                        out=od, op=SUB, in0=od, in1=zv)
                        for occ in range(3):
                            zt = op.tile([128, 16 * 64], F32, tag="zt")
                            nc.scalar.activation(
                                out=zt, in_=acc[:, occ],
                                func=mybir.ActivationFunctionType.Identity,
                                bias=bias_t[:, oh_ * 3 + occ:oh_ * 3 + occ + 1],
                                scale=1.0)
                            lt = op.tile([128, 16 * 64], F32, tag="lt")
                            nc.vector.tensor_scalar_mul(lt, zt, 0.2)
                            ot = op.tile([128, 16 * 64], F32, tag="ot")
                            nc.vector.tensor_tensor(
                                out=ot, in0=zt, in1=lt,
                                op=mybir.AluOpType.max)
                            nc.sync.dma_start(
                                out=out[f, (oh_ * 3 + occ) * 128:
                                        (oh_ * 3 + occ + 1) * 128,
                                        tb * 1024:(tb + 1) * 1024],
                                in_=ot)
    nc.compile()
    return nc


# winograd weight transform (host, weight preprocessing)
_GX = np.array([[1.0, 0.0, 0.0],
                [0.5, 0.5, 0.5],
                [0.5, -0.5, 0.5],
                [0.0, 0.0, 1.0]], dtype=np.float32)

CONV_IMPL = "wino"  # "wino" | "direct"


# ------------------------------------------------------------------- host
def _pack_q(qsd):
    """[nq, d] bf16 -> [128, n_dc*nq] contiguous partition-major tiles."""
    nq, d = qsd.shape
    n_dc = d // 128
    return np.ascontiguousarray(
        qsd.T.reshape(n_dc, 128, nq).transpose(1, 0, 2)).reshape(
            128, n_dc * nq)


def _pack_k(ksd, kbw):
    """[n, d] bf16 -> [n_kbp, n_g, 128, 32*kbw] contiguous K tiles."""
    n, d = ksd.shape
    n_g = d // (32 * 128)
    n_kbp = n // kbw
    kt = ksd.T.reshape(n_g, 32, 128, n_kbp, kbw)
    return np.ascontiguousarray(kt.transpose(3, 0, 2, 1, 4)).reshape(
        n_kbp, n_g, 128, 32 * kbw)


def _windows(z, si, ph, pw):
    """z [bt, c, h, w] -> [b, n, D] for scale si."""
    oh, ow = H // ph, W // pw
    zz = z[:, si * DK:(si + 1) * DK].reshape(B, T, DK, oh, ph, ow, pw)
    zz = zz.transpose(0, 1, 3, 5, 2, 4, 6)
    return np.ascontiguousarray(zz.reshape(B, T * oh * ow, DK * ph * pw))


def _unwindows(y, si, ph, pw):
    """y [b, n, D] -> [bt, DK, h, w] for scale si."""
    oh, ow = H // ph, W // pw
    yy = y.reshape(B, T, oh, ow, DK, ph, pw).transpose(0, 1, 4, 2, 5, 3, 6)
    return yy.reshape(BT, DK, H, W)


def _get(name, builder, *args):
    key = (name,) + args
    if key not in _BUILD_CACHE:
        _BUILD_CACHE[key] = builder(*args)
    return _BUILD_CACHE[key]


def kernel(x, m, wq, bq, wk, bk, wv, bv, wo, bo, b, c):
    x = np.asarray(x, dtype=np.float32)
    assert x.shape == (BT, C, H, W) and int(b) == B and int(c) == C
    cores = list(range(N_CORES))

    # ---- launch A: QKV projections, 2 frames/core
    wT = np.ascontiguousarray(np.concatenate(
        [np.asarray(w)[:, :, 0, 0].T for w in (wq, wk, wv)], axis=1,
        dtype=np.float32)).astype(BF16NP)
    bqkv = np.stack([np.asarray(bq), np.asarray(bk), np.asarray(bv)]
                    ).astype(np.float32)
    x_flat = x.reshape(BT, C, H * W).astype(BF16NP)
    nc_a = _get("proj", _build_proj)
    in_maps = [{"x2": np.ascontiguousarray(
                    x_flat[i * FRAMES_PER_CORE:(i + 1) * FRAMES_PER_CORE]),
                "wT": wT, "bqkv": bqkv} for i in cores]
    res = _run(nc_a, in_maps, cores, "proj")
    qkv = np.concatenate([np.asarray(r["qkv"]) for r in res.results], axis=1)
    q_all = qkv[0].reshape(BT, C, H, W)
    k_all = qkv[1].reshape(BT, C, H, W)
    v_all = qkv[2].reshape(BT, C, H, W)

    # ---- launch B: all scales; scales 1/2 split queries, scale 0 splits
    # V columns (per sample, 4 cores each)
    nc_b = _get("attn", _build_attn)
    in_maps = [dict() for _ in cores]
    for si, (pw_, ph_) in enumerate(PATCHSIZE):
        n, d, nq, d_pv, kbw = _attn_params(si)
        qw = _windows(q_all, si, ph_, pw_)   # [b, n, D] bf16
        kw = _windows(k_all, si, ph_, pw_)
        vw = _windows(v_all, si, ph_, pw_)
        kpk = [_pack_k(kw[s], kbw) for s in range(B)]
        qpk = [_pack_q(qw[s]) for s in range(B)] if si == 0 else None
        for i in cores:
            s, qq = i // 4, i % 4
            if si == 0:
                in_maps[i][f"q{si}"] = qpk[s]
                in_maps[i][f"v{si}"] = np.ascontiguousarray(
                    vw[s][:, qq * d_pv:(qq + 1) * d_pv])
            else:
                in_maps[i][f"q{si}"] = _pack_q(qw[s, qq * nq:(qq + 1) * nq])
                in_maps[i][f"v{si}"] = vw[s]
            in_maps[i][f"k{si}"] = kpk[s]
    res = _run(nc_b, in_maps, cores, "attn")
    y_scales = []
    for si, (pw_, ph_) in enumerate(PATCHSIZE):
        n, d, nq, d_pv, kbw = _attn_params(si)
        y = np.empty((B, n, d), dtype=BF16NP)
        for i in cores:
            s, qq = i // 4, i % 4
            yi = np.asarray(res.results[i][f"y{si}"])
            if si == 0:
                y[s, :, qq * d_pv:(qq + 1) * d_pv] = yi
            else:
                y[s, qq * nq:(qq + 1) * nq] = yi
        y_scales.append(_unwindows(y, si, ph_, pw_))

    y_cat = np.concatenate(y_scales, axis=1)  # [bt, C, h, w] bf16

    # ---- launch C: 3x3 conv + bias + LeakyReLU, 2 frames/core
    y_pad = np.zeros((BT, C, 66, 66), dtype=BF16NP)
    y_pad[:, :, 1:65, 1:65] = y_cat
    y_pad = y_pad.reshape(BT, C, 66 * 66)
    bo_ = np.asarray(bo, dtype=np.float32)
    if CONV_IMPL == "wino":
        # wWx[px*3+dy][i, o] = sum_kx Gx[px,kx] wo[o,i,dy,kx]
        wWx = np.einsum('pk,oidk->pdio', _GX,
                        np.asarray(wo, dtype=np.float32),
                        optimize=True).reshape(12, C, C).astype(BF16NP)
        nc_c = _get("convw", _build_conv_wino)
        in_maps = [{"y2pad": np.ascontiguousarray(
                        y_pad[i * FRAMES_PER_CORE:(i + 1) * FRAMES_PER_CORE]),
                    "wWx": np.ascontiguousarray(wWx), "bo": bo_}
                   for i in cores]
    else:
        woT = np.ascontiguousarray(
            np.asarray(wo, dtype=np.float32).transpose(2, 3, 1, 0)
            .reshape(9, C, C)).astype(BF16NP)
        nc_c = _get("conv", _build_conv)
        in_maps = [{"y2pad": np.ascontiguousarray(
                        y_pad[i * FRAMES_PER_CORE:(i + 1) * FRAMES_PER_CORE]),
                    "woT": woT, "bo": bo_} for i in cores]
    res = _run(nc_c, in_maps, cores, "conv")
    out = np.concatenate([np.asarray(r["out"], dtype=np.float32)
                          for r in res.results], axis=0)
    return out.reshape(BT, C, H, W)


# revision 25
# speedup vs baseline: 1.1282x; 1.0351x over previous
"""Trainium2 Bass kernel for nn_MultiHeadedAttention_9706626089976.

Multi-scale windowed attention over video frames + 3x3 output conv.

v2: 3 SPMD launches on 8 NeuronCores (host does sharding/permutes only):
  A : 1x1-conv QKV projections, data-parallel over the 16 frames (2/core),
      bf16 in/out, N=1024 moving tiles.
  B : all three attention scales in ONE launch; per core = (sample, query
      quarter) for each scale; bf16 transport halves the HBM traffic that
      bounds the small scales.
  C : 3x3 conv + bias + LeakyReLU(0.2), data-parallel over frames (2/core);
      bf16 weights resident in SBUF (loaded once), fused Lrelu activation.

Attention computes scores TRANSPOSED (scoresT[key, q] = K^T-chunks @ Q) so
softmax needs no max-pass/no transposes and exp(scoresT) is directly the
lhsT operand of the P@V matmul.
"""

import hashlib
import math
import os
import shutil

import ml_dtypes
import numpy as np

import concourse.bass as bass
import concourse.bass2jax as bass2jax
import concourse.mybir as mybir
import concourse.tile as tile
from concourse import bacc
from concourse.bass_utils import run_bass_kernel_spmd

BF16NP = ml_dtypes.bfloat16

# Deterministic on-disk NEFF cache keyed on BIR content (walrus compile of
# a launch is minutes; identical BIR always yields the same NEFF).
_NEFF_CACHE_DIR = "/tmp/neff_cache"
_orig_compile_bir_kernel = bass2jax.compile_bir_kernel


def _cached_compile_bir_kernel(bir_json, tmpdir, neff_name="file.neff"):
    data = bir_json if isinstance(bir_json, bytes) else bir_json.encode()
    h = hashlib.sha256(data).hexdigest()
    cpath = os.path.join(_NEFF_CACHE_DIR, h + ".neff")
    if os.path.exists(cpath):
        dst = os.path.join(tmpdir, neff_name)
        shutil.copyfile(cpath, dst)
        return dst
    path = _orig_compile_bir_kernel(bir_json, tmpdir, neff_name=neff_name)
    try:
        os.makedirs(_NEFF_CACHE_DIR, exist_ok=True)
        tmp = cpath + ".tmp." + str(os.getpid())
        shutil.copyfile(path, tmp)
        os.replace(tmp, cpath)
    except OSError:
        pass
    return path


bass2jax.compile_bir_kernel = _cached_compile_bir_kernel

# Problem constants (hardcoded per harness contract).
BT, B, T, C, H, W = 16, 2, 8, 768, 64, 64
DK = 256
FRAMES_PER_CORE = BT // 8
PATCHSIZE = [(16, 16), (8, 8), (4, 4)]
N_CORES = 8

F32 = mybir.dt.float32
BF16 = mybir.dt.bfloat16

# (n, d) per scale; nq = n // 4 (4-way query split per sample).
SCALES = []
for _si, (_pw, _ph) in enumerate(PATCHSIZE):
    _oh, _ow = H // _ph, W // _pw
    SCALES.append((T * _oh * _ow, DK * _ph * _pw))

_BUILD_CACHE = {}

# test.py sets TRACE=True to collect per-launch HW exec times into TIMES.
TRACE = False
TIMES = []


def _run(nc, in_maps, cores, label):
    res = run_bass_kernel_spmd(nc, in_maps, core_ids=cores, trace=TRACE)
    if TRACE:
        TIMES.append((label, res.exec_time_ns))
    return res


def _bacc():
    return bacc.Bacc("TRN2", target_bir_lowering=False, debug=False,
                     num_devices=N_CORES)


# ---------------------------------------------------------------- launch A
def _build_proj():
    """Per core: x2 [2,768,4096] bf16 -> qkv [3,2,768,4096] bf16."""
    nc = _bacc()
    x_in = nc.dram_tensor("x2", [FRAMES_PER_CORE, C, H * W], BF16,
                          kind="ExternalInput").ap()
    w_in = nc.dram_tensor("wT", [C, 3 * C], BF16, kind="ExternalInput").ap()
    b_in = nc.dram_tensor("bqkv", [3, C], F32, kind="ExternalInput").ap()
    out = nc.dram_tensor("qkv", [3, FRAMES_PER_CORE, C, H * W], BF16,
                         kind="ExternalOutput").ap()
    CC = C // 128  # 6 channel chunks
    NB = 512       # moving-dim block (PSUM bank limit: 512 f32)
    n_pb = (H * W) // NB
    with tile.TileContext(nc) as tc:
        with tc.tile_pool(name="wp", bufs=1) as wp, \
             tc.tile_pool(name="xp", bufs=2) as xp, \
             tc.tile_pool(name="op", bufs=4) as op, \
             tc.tile_pool(name="pp", bufs=3, space="PSUM") as pp:
            w_t = wp.tile([128, CC, 3 * C], BF16)
            nc.sync.dma_start(out=w_t, in_=w_in.rearrange("(c k) n -> k c n", k=128))
            bias_t = wp.tile([128, 3, CC], F32)
            nc.sync.dma_start(out=bias_t,
                              in_=b_in.rearrange("p (c k) -> k p c", k=128))
            for f in range(FRAMES_PER_CORE):
                x_t = xp.tile([128, CC, H * W], BF16)
                nc.sync.dma_start(
                    out=x_t, in_=x_in[f].rearrange("(c k) p -> k c p", k=128))
                for p in range(3):
                    for oc in range(CC):
                        for pb in range(n_pb):
                            ps = pp.tile([128, NB], F32)
                            for ic in range(CC):
                                nc.tensor.matmul(
                                    ps,
                                    w_t[:, ic, p * C + oc * 128:p * C + oc * 128 + 128],
                                    x_t[:, ic, pb * NB:(pb + 1) * NB],
                                    start=(ic == 0), stop=(ic == CC - 1))
                            ot = op.tile([128, NB], BF16)
                            nc.scalar.activation(
                                out=ot, in_=ps,
                                func=mybir.ActivationFunctionType.Identity,
                                bias=bias_t[:, p, oc:oc + 1], scale=1.0)
                            nc.sync.dma_start(
                                out=out[p, f, oc * 128:(oc + 1) * 128,
                                        pb * NB:(pb + 1) * NB],
                                in_=ot)
    nc.compile()
    return nc


# ---------------------------------------------------------------- launch B
def _emit_attn_scale(nc, pools, q_in, k_in, v_in, y_out, n, d, nq, d_pv, kbw):
    """Emit one scale's windowed attention. Per core:
      Q packed [128, n_dc*nq] bf16, K packed [n_kbp, n_g, 128, DCG*kbw]
      bf16 (host pre-tiled so every DMA is fully contiguous),
      V [n, d_pv] bf16 -> y [nq, d_pv] bf16.
    scoresT[key, q] accumulated in PSUM over d; exp on ACT (scale folded);
    key-sums via ones-matmul; P@V with expT as lhsT; normalization folded
    into the PSUM->SBUF copy of y. d_pv < d means this core only computes
    a column-slice of y (scale-0: full queries, quarter of V columns).
    Pools are shared across scales (padded tiles, fixed tags) so buffer
    rotation serializes memory reuse."""
    qp, kp, ep, vp, yp, sp, pp, py, pq = pools
    scale = 1.0 / math.sqrt(d)
    n_kb = n // 128           # key blocks
    n_dc = d // 128           # contraction chunks
    DCG = 32                  # d-chunks per streamed K group (4096 rows)
    n_g = n_dc // DCG
    n_sub = kbw // 128        # key blocks per packed K load
    n_qb = max(1, nq // 128)  # query blocks (nq may be < 128)
    dj_cols = 512             # V column block
    n_dj = d_pv // dj_cols

    v_r = v_in.rearrange("(c k) e -> k c e", k=128)

    # Q resident when it fits; else (scale 0: full queries x full d) stream
    # it group-wise like K — safe since each chunk is read once (n_kb == 1).
    stream_q = n_dc * nq > 16384
    if stream_q:
        assert n_kb == n_sub == 1
        q_v = None
    else:
        q_t = qp.tile([128, 16384], BF16, tag="q")
        q_v = q_t[:, :n_dc * nq].rearrange("k (c n) -> k c n", n=nq)
        nc.sync.dma_start(out=q_v,
                          in_=q_in.rearrange("k (c n) -> k c n", n=nq))
    ones_t = sp.tile([128, 2], BF16, tag="one")
    nc.vector.memset(ones_t, 1.0)
    exp_t = ep.tile([128, 8192], BF16, tag="e")
    exp_v = exp_t[:, :n_kb * nq].rearrange("k (b n) -> k b n", n=nq)

    for kbp in range(n_kb // n_sub):
        st_list = [pp.tile([128, 512], F32, tag="s", name=f"st{sub}")
                   for sub in range(n_sub)]
        for g in range(n_g):
            k_t = kp.tile([128, DCG * kbw], BF16, tag="k")
            nc.sync.dma_start(out=k_t, in_=k_in[kbp, g])
            k_v = k_t.rearrange("k (c n) -> k c n", c=DCG)
            if stream_q:
                q_g = qp.tile([128, DCG * nq], BF16, tag="qg")
                nc.sync.dma_start(
                    out=q_g,
                    in_=q_in[:, g * DCG * nq:(g + 1) * DCG * nq])
                q_gv = q_g.rearrange("k (c n) -> k c n", n=nq)
            for sub in range(n_sub):
                for c_ in range(DCG):
                    dc = g * DCG + c_
                    nc.tensor.matmul(
                        st_list[sub][:, :nq],
                        k_v[:, c_, sub * 128:(sub + 1) * 128],
                        q_gv[:, c_, :] if stream_q else q_v[:, dc, :],
                        start=(dc == 0), stop=(dc == n_dc - 1))
        for sub in range(n_sub):
            kb = kbp * n_sub + sub
            nc.scalar.activation(out=exp_v[:, kb, :],
                                 in_=st_list[sub][:, :nq],
                                 func=mybir.ActivationFunctionType.Exp,
                                 scale=scale)
    # per-query key-sums, partition-oriented: sums[q] over keys.
    sums_ps = pq.tile([128, 8], F32, tag="sm")
    for qb in range(n_qb):
        mq = min(128, nq - qb * 128)
        for kb in range(n_kb):
            nc.tensor.matmul(
                sums_ps[:mq, 2 * qb:2 * qb + 2],
                exp_v[:, kb, qb * 128:qb * 128 + mq],
                ones_t[:, 0:2],
                start=(kb == 0), stop=(kb == n_kb - 1))
    mq0 = min(128, nq)
    rq_t = sp.tile([128, 4], F32, tag="r")
    nc.vector.reciprocal(
        out=rq_t[:mq0, :n_qb],
        in_=sums_ps.rearrange("k (b two) -> k b two", two=2)[:mq0, :n_qb, 0])

    for dj in range(n_dj):
        v_t = vp.tile([128, 8192], BF16, tag="v")
        v_v = v_t.rearrange("k (b e) -> k b e", e=dj_cols)
        nc.sync.dma_start(out=v_v[:, :n_kb, :],
                          in_=v_r[:, :, dj * dj_cols:(dj + 1) * dj_cols])
        for qb in range(n_qb):
            mq = min(128, nq - qb * 128)
            y_ps = py.tile([128, dj_cols], F32, tag="y")
            for kb in range(n_kb):
                nc.tensor.matmul(
                    y_ps[:mq, :],
                    exp_v[:, kb, qb * 128:qb * 128 + mq],
                    v_v[:, kb, :],
                    start=(kb == 0), stop=(kb == n_kb - 1))
            y_t = yp.tile([128, dj_cols], BF16, tag="o")
            nc.vector.tensor_scalar_mul(
                y_t[:mq, :], y_ps[:mq, :], rq_t[:mq, qb:qb + 1])
            nc.sync.dma_start(
                out=y_out[qb * 128:qb * 128 + mq,
                          dj * dj_cols:(dj + 1) * dj_cols],
                in_=y_t[:mq, :])


def _attn_params(si):
    """(n, d, nq, d_pv, kbw) for scale si. Scale 0: full queries per core,
    V-column quarter (its n=128 makes query-splitting dispatch-bound);
    scales 1/2: query quarter, full V columns. kbw = keys per packed K
    load (256 gives 512B+ contiguous DMA runs)."""
    n, d = SCALES[si]
    if si == 0:
        return n, d, n, d // 4, 128
    return n, d, n // 4, d, 256


def _build_attn():
    """One launch, all 3 scales. Per core = (sample, query quarter)."""
    nc = _bacc()
    ins, outs = [], []
    for si in range(3):
        n, d, nq, d_pv, kbw = _attn_params(si)
        n_dc = d // 128
        n_g = n_dc // 32
        n_kbp = n // kbw
        ins.append((
            nc.dram_tensor(f"q{si}", [128, n_dc * nq], BF16,
                           kind="ExternalInput").ap(),
            nc.dram_tensor(f"k{si}", [n_kbp, n_g, 128, 32 * kbw], BF16,
                           kind="ExternalInput").ap(),
            nc.dram_tensor(f"v{si}", [n, d_pv], BF16,
                           kind="ExternalInput").ap(),
        ))
        outs.append(
            nc.dram_tensor(f"y{si}", [nq, d_pv], BF16,
                           kind="ExternalOutput").ap())
    with tile.TileContext(nc) as tc:
        with tc.tile_pool(name="qp", bufs=2) as qp, \
             tc.tile_pool(name="kp", bufs=3) as kp, \
             tc.tile_pool(name="ep", bufs=2) as ep, \
             tc.tile_pool(name="vp", bufs=2) as vp, \
             tc.tile_pool(name="yp", bufs=4) as yp, \
             tc.tile_pool(name="sp", bufs=2) as sp, \
             tc.tile_pool(name="pp", bufs=3, space="PSUM") as pp, \
             tc.tile_pool(name="py", bufs=3, space="PSUM") as py, \
             tc.tile_pool(name="pq", bufs=2, space="PSUM") as pq:
            pools = (qp, kp, ep, vp, yp, sp, pp, py, pq)
            # big scale first: its long QK phase overlaps later scales' DMA
            for si in (2, 1, 0):
                n, d, nq, d_pv, kbw = _attn_params(si)
                q_in, k_in, v_in = ins[si]
                _emit_attn_scale(nc, pools, q_in, k_in, v_in, outs[si],
                                 n, d, nq, d_pv, kbw)
    nc.compile()
    return nc


# ---------------------------------------------------------------- launch C
def _build_conv():
    """Per core: y2pad [2,768,66,66] bf16, woT [9,768,768] bf16, bo [768]
    -> out [2,768,4096] f32 with bias + LeakyReLU(0.2)."""
    nc = _bacc()
    x_in = nc.dram_tensor("y2pad", [FRAMES_PER_CORE, C, 66 * 66], BF16,
                          kind="ExternalInput").ap()
    w_in = nc.dram_tensor("woT", [9, C, C], BF16, kind="ExternalInput").ap()
    b_in = nc.dram_tensor("bo", [C], F32, kind="ExternalInput").ap()
    out = nc.dram_tensor("out", [FRAMES_PER_CORE, C, H * W], F32,
                         kind="ExternalOutput").ap()
    CC = C // 128
    NR = 8  # output rows per block (N = NR*64 = 512, PSUM bank limit)
    n_rb = H // NR
    with tile.TileContext(nc) as tc:
        with tc.tile_pool(name="wp", bufs=1) as wp, \
             tc.tile_pool(name="xp", bufs=2) as xp, \
             tc.tile_pool(name="op", bufs=3) as op, \
             tc.tile_pool(name="pp", bufs=3, space="PSUM") as pp:
            # all weights resident: [128(ic%128), 9, CC(ic//128), 768(oc)]
            w_t = wp.tile([128, 9, CC, C], BF16)
            nc.sync.dma_start(
                out=w_t, in_=w_in.rearrange("s (c k) o -> k s c o", k=128))
            bias_t = wp.tile([128, CC], F32)
            nc.sync.dma_start(out=bias_t,
                              in_=b_in.rearrange("(c k) -> k c", k=128))
            for f in range(FRAMES_PER_CORE):
                x_t = xp.tile([128, CC, 66 * 66], BF16)
                nc.sync.dma_start(
                    out=x_t, in_=x_in[f].rearrange("(c k) p -> k c p", k=128))
                x_v = x_t.rearrange("k c (r q) -> k c r q", r=66)
                for oc in range(CC):
                    for rb in range(n_rb):
                        ps = pp.tile([128, NR * 64], F32)
                        first = True
                        for dy in range(3):
                            for dx in range(3):
                                for ic in range(CC):
                                    y0 = rb * NR + dy
                                    rhs = x_v[:, ic, y0:y0 + NR, dx:dx + 64]
                                    nc.tensor.matmul(
                                        ps,
                                        w_t[:, dy * 3 + dx, ic,
                                            oc * 128:(oc + 1) * 128],
                                        rhs,
                                        start=first,
                                        stop=(dy == 2 and dx == 2 and ic == CC - 1))
                                    first = False
                        zt = op.tile([128, NR * 64], F32, tag="zt")
                        nc.scalar.activation(
                            out=zt, in_=ps,
                            func=mybir.ActivationFunctionType.Identity,
                            bias=bias_t[:, oc:oc + 1], scale=1.0)
                        lt = op.tile([128, NR * 64], F32, tag="lt")
                        nc.vector.tensor_scalar_mul(lt, zt, 0.2)
                        ot = op.tile([128, NR * 64], F32, tag="ot")
                        nc.vector.tensor_tensor(
                            out=ot, in0=zt, in1=lt, op=mybir.AluOpType.max)
                        nc.sync.dma_start(
                            out=out[f, oc * 128:(oc + 1) * 128,
                                    rb * (NR * 64):(rb + 1) * (NR * 64)],
                            in_=ot)
    nc.compile()
    return nc


def _build_conv_wino():
    """1D (width) Winograd F(2,3) conv: 1.5x fewer MACs than direct.
    Per core: y2pad [2,768,66,66] bf16, wWx [12,768,768] bf16 (px*3+dy,
    ic, oc = G-transformed weights), bo [768] f32 -> out [2,768,4096] f32.

    Per 16-output-row batch: T1 = B^T-combine of input cols (4 px slices,
    DVE); per (px, oc-chunk): PSUM accumulates sum_dy sum_ic W~[px,dy]^T @
    T1[rows+dy]; DVE A^T-combines the 4 px results into even/odd output
    columns; ACT applies bias + LeakyReLU."""
    nc = _bacc()
    x_in = nc.dram_tensor("y2pad", [FRAMES_PER_CORE, C, 66 * 66], BF16,
                          kind="ExternalInput").ap()
    w_in = nc.dram_tensor("wWx", [12, C, C], BF16, kind="ExternalInput").ap()
    b_in = nc.dram_tensor("bo", [C], F32, kind="ExternalInput").ap()
    out = nc.dram_tensor("out", [FRAMES_PER_CORE, C, H * W], F32,
                         kind="ExternalOutput").ap()
    CC = C // 128
    ADD, SUB = mybir.AluOpType.add, mybir.AluOpType.subtract
    with tile.TileContext(nc) as tc:
        with tc.tile_pool(name="wp", bufs=1) as wp, \
             tc.tile_pool(name="xp", bufs=1) as xp, \
             tc.tile_pool(name="tp", bufs=1) as tp, \
             tc.tile_pool(name="ap", bufs=1) as acp, \
             tc.tile_pool(name="op", bufs=2) as op, \
             tc.tile_pool(name="bp", bufs=1) as bp, \
             tc.tile_pool(name="pp", bufs=3, space="PSUM") as pp:
            bias_t = bp.tile([128, CC], F32)
            nc.sync.dma_start(out=bias_t,
                              in_=b_in.rearrange("(c k) -> k c", k=128))
            # all 12 transformed-weight matrices resident (one pass over x;
            # chunked loads so the first matmuls start after chunk 0)
            w_t = wp.tile([128, 12, CC, C], BF16)
            w_r = w_in.rearrange("s (c k) o -> k s c o", k=128)
            for s in range(12):
                nc.sync.dma_start(out=w_t[:, s], in_=w_r[:, s])
            for f in range(FRAMES_PER_CORE):
                for tb in range(4):  # 16-output-row batches
                    y0 = tb * 16
                    x_t = xp.tile([128, CC, 18 * 66], BF16, tag="x")
                    nc.sync.dma_start(
                        out=x_t,
                        in_=x_in[f][:, y0 * 66:(y0 + 18) * 66].rearrange(
                            "(c k) p -> k c p", k=128))
                    # xe[..., t, 0] = col 2t, xe[..., t, 1] = col 2t+1
                    xe = x_t.rearrange("k c (r t two) -> k c r t two",
                                       two=2, t=33)
                    t1 = tp.tile([128, CC, 18, 128], BF16, tag="t1")
                    t1v = t1.rearrange("k c r (p t) -> k c r p t", p=4)
                    # u0=d0-d2, u1=d1+d2, u2=d2-d1, u3=d1-d3
                    nc.vector.tensor_tensor(
                        out=t1v[:, :, :, 0, :], op=SUB,
                        in0=xe[:, :, :, 0:32, 0], in1=xe[:, :, :, 1:33, 0])
                    nc.vector.tensor_tensor(
                        out=t1v[:, :, :, 1, :], op=ADD,
                        in0=xe[:, :, :, 0:32, 1], in1=xe[:, :, :, 1:33, 0])
                    nc.vector.tensor_tensor(
                        out=t1v[:, :, :, 2, :], op=SUB,
                        in0=xe[:, :, :, 1:33, 0], in1=xe[:, :, :, 0:32, 1])
                    nc.vector.tensor_tensor(
                        out=t1v[:, :, :, 3, :], op=SUB,
                        in0=xe[:, :, :, 0:32, 1], in1=xe[:, :, :, 1:33, 1])
                    acc = acp.tile([128, CC, 16, 64], F32, tag="acc")
                    accv = acc.rearrange("k c r (t two) -> k c r t two",
                                         two=2)
                    for px in range(4):
                        for occ in range(CC):
                            psz = pp.tile([128, 512], F32, tag="z")
                            first = True
                            for dy in range(3):
                                for ic in range(CC):
                                    nc.tensor.matmul(
                                        psz,
                                        w_t[:, px * 3 + dy, ic,
                                            occ * 128:(occ + 1) * 128],
                                        t1v[:, ic, dy:dy + 16, px, :],
                                        start=first,
                                        stop=(dy == 2 and ic == CC - 1))
                                    first = False
                            zv = psz.rearrange("k (r t) -> k r t", r=16)
                            ev = accv[:, occ, :, :, 0]
                            od = accv[:, occ, :, :, 1]
                            # A^T: even = z0+z1+z2 ; odd = z1-z2-z3
                            if px == 0:
                                nc.vector.tensor_copy(out=ev, in_=zv)
                            elif px == 1:
                                nc.vector.tensor_tensor(
                                    out=ev, op=ADD, in0=ev, in1=zv)
                                nc.vector.tensor_copy(out=od, in_=zv)
                            elif px == 2:
                                nc.vector.tensor_tensor(
                                    out=ev, op=ADD, in0=ev, in1=zv)
                                nc.vector.tensor_tensor(
                                    out=od, op=SUB, in0=od, in1=zv)
                            else:
                                nc.vector.tensor_tensor(
                                    out=od, op=SUB, in0=od, in1=zv)
                    for occ in range(CC):
                        zt = op.tile([128, 16 * 64], F32, tag="zt")
                        nc.scalar.activation(
                            out=zt, in_=acc[:, occ],
                            func=mybir.ActivationFunctionType.Identity,
                            bias=bias_t[:, occ:occ + 1], scale=1.0)
                        lt = op.tile([128, 16 * 64], F32, tag="lt")
                        nc.vector.tensor_scalar_mul(lt, zt, 0.2)
                        ot = op.tile([128, 16 * 64], F32, tag="ot")
                        nc.vector.tensor_tensor(
                            out=ot, in0=zt, in1=lt,
                            op=mybir.AluOpType.max)
                        nc.sync.dma_start(
                            out=out[f, occ * 128:(occ + 1) * 128,
                                    tb * 1024:(tb + 1) * 1024],
                            in_=ot)
    nc.compile()
    return nc


# winograd weight transform (host, weight preprocessing)
_GX = np.array([[1.0, 0.0, 0.0],
                [0.5, 0.5, 0.5],
                [0.5, -0.5, 0.5],
                [0.0, 0.0, 1.0]], dtype=np.float32)

CONV_IMPL = "wino"  # "wino" | "direct"


# ------------------------------------------------------------------- host
def _pack_q(qsd):
    """[nq, d] bf16 -> [128, n_dc*nq] contiguous partition-major tiles."""
    nq, d = qsd.shape
    n_dc = d // 128
    return np.ascontiguousarray(
        qsd.T.reshape(n_dc, 128, nq).transpose(1, 0, 2)).reshape(
            128, n_dc * nq)


def _pack_k(ksd, kbw):
    """[n, d] bf16 -> [n_kbp, n_g, 128, 32*kbw] contiguous K tiles."""
    n, d = ksd.shape
    n_g = d // (32 * 128)
    n_kbp = n // kbw
    kt = ksd.T.reshape(n_g, 32, 128, n_kbp, kbw)
    return np.ascontiguousarray(kt.transpose(3, 0, 2, 1, 4)).reshape(
        n_kbp, n_g, 128, 32 * kbw)


def _windows(z, si, ph, pw):
    """z [bt, c, h, w] -> [b, n, D] for scale si."""
    oh, ow = H // ph, W // pw
    zz = z[:, si * DK:(si + 1) * DK].reshape(B, T, DK, oh, ph, ow, pw)
    zz = zz.transpose(0, 1, 3, 5, 2, 4, 6)
    return np.ascontiguousarray(zz.reshape(B, T * oh * ow, DK * ph * pw))


def _unwindows(y, si, ph, pw):
    """y [b, n, D] -> [bt, DK, h, w] for scale si."""
    oh, ow = H // ph, W // pw
    yy = y.reshape(B, T, oh, ow, DK, ph, pw).transpose(0, 1, 4, 2, 5, 3, 6)
    return yy.reshape(BT, DK, H, W)


def _get(name, builder, *args):
    key = (name,) + args
    if key not in _BUILD_CACHE:
        _BUILD_CACHE[key] = builder(*args)
    return _BUILD_CACHE[key]


def kernel(x, m, wq, bq, wk, bk, wv, bv, wo, bo, b, c):
    x = np.asarray(x, dtype=np.float32)
    assert x.shape == (BT, C, H, W) and int(b) == B and int(c) == C
    cores = list(range(N_CORES))

    # ---- launch A: QKV projections, 2 frames/core
    wT = np.ascontiguousarray(np.concatenate(
        [np.asarray(w)[:, :, 0, 0].T for w in (wq, wk, wv)], axis=1,
        dtype=np.float32)).astype(BF16NP)
    bqkv = np.stack([np.asarray(bq), np.asarray(bk), np.asarray(bv)]
                    ).astype(np.float32)
    x_flat = x.reshape(BT, C, H * W).astype(BF16NP)
    nc_a = _get("proj", _build_proj)
    in_maps = [{"x2": np.ascontiguousarray(
                    x_flat[i * FRAMES_PER_CORE:(i + 1) * FRAMES_PER_CORE]),
                "wT": wT, "bqkv": bqkv} for i in cores]
    res = _run(nc_a, in_maps, cores, "proj")
    qkv = np.concatenate([np.asarray(r["qkv"]) for r in res.results], axis=1)
    q_all = qkv[0].reshape(BT, C, H, W)
    k_all = qkv[1].reshape(BT, C, H, W)
    v_all = qkv[2].reshape(BT, C, H, W)

    # ---- launch B: all scales; scales 1/2 split queries, scale 0 splits
    # V columns (per sample, 4 cores each)
    nc_b = _get("attn", _build_attn)
    in_maps = [dict() for _ in cores]
    for si, (pw_, ph_) in enumerate(PATCHSIZE):
        n, d, nq, d_pv, kbw = _attn_params(si)
        qw = _windows(q_all, si, ph_, pw_)   # [b, n, D] bf16
        kw = _windows(k_all, si, ph_, pw_)
        vw = _windows(v_all, si, ph_, pw_)
        kpk = [_pack_k(kw[s], kbw) for s in range(B)]
        qpk = [_pack_q(qw[s]) for s in range(B)] if si == 0 else None
        for i in cores:
            s, qq = i // 4, i % 4
            if si == 0:
                in_maps[i][f"q{si}"] = qpk[s]
                in_maps[i][f"v{si}"] = np.ascontiguousarray(
                    vw[s][:, qq * d_pv:(qq + 1) * d_pv])
            else:
                in_maps[i][f"q{si}"] = _pack_q(qw[s, qq * nq:(qq + 1) * nq])
                in_maps[i][f"v{si}"] = vw[s]
            in_maps[i][f"k{si}"] = kpk[s]
    res = _run(nc_b, in_maps, cores, "attn")
    y_scales = []
    for si, (pw_, ph_) in enumerate(PATCHSIZE):
        n, d, nq, d_pv, kbw = _attn_params(si)
        y = np.empty((B, n, d), dtype=BF16NP)
        for i in cores:
            s, qq = i // 4, i % 4
            yi = np.asarray(res.results[i][f"y{si}"])
            if si == 0:
                y[s, :, qq * d_pv:(qq + 1) * d_pv] = yi
            else:
                y[s, qq * nq:(qq + 1) * nq] = yi
        y_scales.append(_unwindows(y, si, ph_, pw_))

    y_cat = np.concatenate(y_scales, axis=1)  # [bt, C, h, w] bf16

    # ---- launch C: 3x3 conv + bias + LeakyReLU, 2 frames/core
    y_pad = np.zeros((BT, C, 66, 66), dtype=BF16NP)
    y_pad[:, :, 1:65, 1:65] = y_cat
    y_pad = y_pad.reshape(BT, C, 66 * 66)
    bo_ = np.asarray(bo, dtype=np.float32)
    if CONV_IMPL == "wino":
        # wWx[px*3+dy][i, o] = sum_kx Gx[px,kx] wo[o,i,dy,kx]
        wWx = np.einsum('pk,oidk->pdio', _GX,
                        np.asarray(wo, dtype=np.float32),
                        optimize=True).reshape(12, C, C).astype(BF16NP)
        nc_c = _get("convw", _build_conv_wino)
        in_maps = [{"y2pad": np.ascontiguousarray(
                        y_pad[i * FRAMES_PER_CORE:(i + 1) * FRAMES_PER_CORE]),
                    "wWx": np.ascontiguousarray(wWx), "bo": bo_}
                   for i in cores]
    else:
        woT = np.ascontiguousarray(
            np.asarray(wo, dtype=np.float32).transpose(2, 3, 1, 0)
            .reshape(9, C, C)).astype(BF16NP)
        nc_c = _get("conv", _build_conv)
        in_maps = [{"y2pad": np.ascontiguousarray(
                        y_pad[i * FRAMES_PER_CORE:(i + 1) * FRAMES_PER_CORE]),
                    "woT": woT, "bo": bo_} for i in cores]
    res = _run(nc_c, in_maps, cores, "conv")
    out = np.concatenate([np.asarray(r["out"], dtype=np.float32)
                          for r in res.results], axis=0)
    return out.reshape(BT, C, H, W)


# revision 26
# speedup vs baseline: 1.1382x; 1.0088x over previous
"""Trainium2 Bass kernel for nn_MultiHeadedAttention_9706626089976.

Multi-scale windowed attention over video frames + 3x3 output conv.

v2: 3 SPMD launches on 8 NeuronCores (host does sharding/permutes only):
  A : 1x1-conv QKV projections, data-parallel over the 16 frames (2/core),
      bf16 in/out, N=1024 moving tiles.
  B : all three attention scales in ONE launch; per core = (sample, query
      quarter) for each scale; bf16 transport halves the HBM traffic that
      bounds the small scales.
  C : 3x3 conv + bias + LeakyReLU(0.2), data-parallel over frames (2/core);
      bf16 weights resident in SBUF (loaded once), fused Lrelu activation.

Attention computes scores TRANSPOSED (scoresT[key, q] = K^T-chunks @ Q) so
softmax needs no max-pass/no transposes and exp(scoresT) is directly the
lhsT operand of the P@V matmul.
"""

import hashlib
import math
import os
import shutil

import ml_dtypes
import numpy as np

import concourse.bass as bass
import concourse.bass2jax as bass2jax
import concourse.mybir as mybir
import concourse.tile as tile
from concourse import bacc
from concourse.bass_utils import run_bass_kernel_spmd

BF16NP = ml_dtypes.bfloat16

# Deterministic on-disk NEFF cache keyed on BIR content (walrus compile of
# a launch is minutes; identical BIR always yields the same NEFF).
_NEFF_CACHE_DIR = "/tmp/neff_cache"
_orig_compile_bir_kernel = bass2jax.compile_bir_kernel


def _cached_compile_bir_kernel(bir_json, tmpdir, neff_name="file.neff"):
    data = bir_json if isinstance(bir_json, bytes) else bir_json.encode()
    h = hashlib.sha256(data).hexdigest()
    cpath = os.path.join(_NEFF_CACHE_DIR, h + ".neff")
    if os.path.exists(cpath):
        dst = os.path.join(tmpdir, neff_name)
        shutil.copyfile(cpath, dst)
        return dst
    path = _orig_compile_bir_kernel(bir_json, tmpdir, neff_name=neff_name)
    try:
        os.makedirs(_NEFF_CACHE_DIR, exist_ok=True)
        tmp = cpath + ".tmp." + str(os.getpid())
        shutil.copyfile(path, tmp)
        os.replace(tmp, cpath)
    except OSError:
        pass
    return path


bass2jax.compile_bir_kernel = _cached_compile_bir_kernel

# Problem constants (hardcoded per harness contract).
BT, B, T, C, H, W = 16, 2, 8, 768, 64, 64
DK = 256
FRAMES_PER_CORE = BT // 8
PATCHSIZE = [(16, 16), (8, 8), (4, 4)]
N_CORES = 8

F32 = mybir.dt.float32
BF16 = mybir.dt.bfloat16

# (n, d) per scale; nq = n // 4 (4-way query split per sample).
SCALES = []
for _si, (_pw, _ph) in enumerate(PATCHSIZE):
    _oh, _ow = H // _ph, W // _pw
    SCALES.append((T * _oh * _ow, DK * _ph * _pw))

_BUILD_CACHE = {}

# test.py sets TRACE=True to collect per-launch HW exec times into TIMES.
TRACE = False
TIMES = []


def _run(nc, in_maps, cores, label):
    res = run_bass_kernel_spmd(nc, in_maps, core_ids=cores, trace=TRACE)
    if TRACE:
        TIMES.append((label, res.exec_time_ns))
    return res


def _bacc():
    return bacc.Bacc("TRN2", target_bir_lowering=False, debug=False,
                     num_devices=N_CORES)


# ---------------------------------------------------------------- launch A
def _build_proj():
    """Per core: x2 [2,768,4096] bf16 -> qkv [3,2,768,4096] bf16."""
    nc = _bacc()
    x_in = nc.dram_tensor("x2", [FRAMES_PER_CORE, C, H * W], BF16,
                          kind="ExternalInput").ap()
    w_in = nc.dram_tensor("wT", [C, 3 * C], BF16, kind="ExternalInput").ap()
    b_in = nc.dram_tensor("bqkv", [3, C], F32, kind="ExternalInput").ap()
    out = nc.dram_tensor("qkv", [3, FRAMES_PER_CORE, C, H * W], BF16,
                         kind="ExternalOutput").ap()
    CC = C // 128  # 6 channel chunks
    NB = 512       # moving-dim block (PSUM bank limit: 512 f32)
    n_pb = (H * W) // NB
    with tile.TileContext(nc) as tc:
        with tc.tile_pool(name="wp", bufs=1) as wp, \
             tc.tile_pool(name="xp", bufs=2) as xp, \
             tc.tile_pool(name="op", bufs=4) as op, \
             tc.tile_pool(name="pp", bufs=3, space="PSUM") as pp:
            # chunked first loads (region-level deps): the first matmul
            # needs only w[:, :, :C] and x[:, :, :NB], not the full 10 MB
            w_t = wp.tile([128, CC, 3 * C], BF16)
            w_r = w_in.rearrange("(c k) n -> k c n", k=128)
            for p in range(3):
                nc.sync.dma_start(out=w_t[:, :, p * C:(p + 1) * C],
                                  in_=w_r[:, :, p * C:(p + 1) * C])
            bias_t = wp.tile([128, 3, CC], F32)
            nc.sync.dma_start(out=bias_t,
                              in_=b_in.rearrange("p (c k) -> k p c", k=128))
            for f in range(FRAMES_PER_CORE):
                x_t = xp.tile([128, CC, H * W], BF16)
                x_r = x_in[f].rearrange("(c k) p -> k c p", k=128)
                for pb in range(n_pb):
                    nc.sync.dma_start(
                        out=x_t[:, :, pb * NB:(pb + 1) * NB],
                        in_=x_r[:, :, pb * NB:(pb + 1) * NB])
                for p in range(3):
                    for oc in range(CC):
                        for pb in range(n_pb):
                            ps = pp.tile([128, NB], F32)
                            for ic in range(CC):
                                nc.tensor.matmul(
                                    ps,
                                    w_t[:, ic, p * C + oc * 128:p * C + oc * 128 + 128],
                                    x_t[:, ic, pb * NB:(pb + 1) * NB],
                                    start=(ic == 0), stop=(ic == CC - 1))
                            ot = op.tile([128, NB], BF16)
                            nc.scalar.activation(
                                out=ot, in_=ps,
                                func=mybir.ActivationFunctionType.Identity,
                                bias=bias_t[:, p, oc:oc + 1], scale=1.0)
                            nc.sync.dma_start(
                                out=out[p, f, oc * 128:(oc + 1) * 128,
                                        pb * NB:(pb + 1) * NB],
                                in_=ot)
    nc.compile()
    return nc


# ---------------------------------------------------------------- launch B
def _emit_attn_scale(nc, pools, q_in, k_in, v_in, y_out, n, d, nq, d_pv, kbw):
    """Emit one scale's windowed attention. Per core:
      Q packed [128, n_dc*nq] bf16, K packed [n_kbp, n_g, 128, DCG*kbw]
      bf16 (host pre-tiled so every DMA is fully contiguous),
      V [n, d_pv] bf16 -> y [nq, d_pv] bf16.
    scoresT[key, q] accumulated in PSUM over d; exp on ACT (scale folded);
    key-sums via ones-matmul; P@V with expT as lhsT; normalization folded
    into the PSUM->SBUF copy of y. d_pv < d means this core only computes
    a column-slice of y (scale-0: full queries, quarter of V columns).
    Pools are shared across scales (padded tiles, fixed tags) so buffer
    rotation serializes memory reuse."""
    qp, kp, ep, vp, yp, sp, pp, py, pq = pools
    scale = 1.0 / math.sqrt(d)
    n_kb = n // 128           # key blocks
    n_dc = d // 128           # contraction chunks
    DCG = 32                  # d-chunks per streamed K group (4096 rows)
    n_g = n_dc // DCG
    n_sub = kbw // 128        # key blocks per packed K load
    n_qb = max(1, nq // 128)  # query blocks (nq may be < 128)
    dj_cols = 512             # V column block
    n_dj = d_pv // dj_cols

    v_r = v_in.rearrange("(c k) e -> k c e", k=128)

    # Q resident when it fits; else (scale 0: full queries x full d) stream
    # it group-wise like K — safe since each chunk is read once (n_kb == 1).
    stream_q = n_dc * nq > 16384
    if stream_q:
        assert n_kb == n_sub == 1
        q_v = None
    else:
        q_t = qp.tile([128, 16384], BF16, tag="q")
        q_v = q_t[:, :n_dc * nq].rearrange("k (c n) -> k c n", n=nq)
        nc.sync.dma_start(out=q_v,
                          in_=q_in.rearrange("k (c n) -> k c n", n=nq))
    ones_t = sp.tile([128, 2], BF16, tag="one")
    nc.vector.memset(ones_t, 1.0)
    exp_t = ep.tile([128, 8192], BF16, tag="e")
    exp_v = exp_t[:, :n_kb * nq].rearrange("k (b n) -> k b n", n=nq)

    for kbp in range(n_kb // n_sub):
        st_list = [pp.tile([128, 512], F32, tag="s", name=f"st{sub}")
                   for sub in range(n_sub)]
        for g in range(n_g):
            k_t = kp.tile([128, DCG * kbw], BF16, tag="k")
            nc.sync.dma_start(out=k_t, in_=k_in[kbp, g])
            k_v = k_t.rearrange("k (c n) -> k c n", c=DCG)
            if stream_q:
                q_g = qp.tile([128, DCG * nq], BF16, tag="qg")
                nc.sync.dma_start(
                    out=q_g,
                    in_=q_in[:, g * DCG * nq:(g + 1) * DCG * nq])
                q_gv = q_g.rearrange("k (c n) -> k c n", n=nq)
            for sub in range(n_sub):
                for c_ in range(DCG):
                    dc = g * DCG + c_
                    nc.tensor.matmul(
                        st_list[sub][:, :nq],
                        k_v[:, c_, sub * 128:(sub + 1) * 128],
                        q_gv[:, c_, :] if stream_q else q_v[:, dc, :],
                        start=(dc == 0), stop=(dc == n_dc - 1))
        for sub in range(n_sub):
            kb = kbp * n_sub + sub
            nc.scalar.activation(out=exp_v[:, kb, :],
                                 in_=st_list[sub][:, :nq],
                                 func=mybir.ActivationFunctionType.Exp,
                                 scale=scale)
    # per-query key-sums, partition-oriented: sums[q] over keys.
    sums_ps = pq.tile([128, 8], F32, tag="sm")
    for qb in range(n_qb):
        mq = min(128, nq - qb * 128)
        for kb in range(n_kb):
            nc.tensor.matmul(
                sums_ps[:mq, 2 * qb:2 * qb + 2],
                exp_v[:, kb, qb * 128:qb * 128 + mq],
                ones_t[:, 0:2],
                start=(kb == 0), stop=(kb == n_kb - 1))
    mq0 = min(128, nq)
    rq_t = sp.tile([128, 4], F32, tag="r")
    nc.vector.reciprocal(
        out=rq_t[:mq0, :n_qb],
        in_=sums_ps.rearrange("k (b two) -> k b two", two=2)[:mq0, :n_qb, 0])

    for dj in range(n_dj):
        v_t = vp.tile([128, 8192], BF16, tag="v")
        v_v = v_t.rearrange("k (b e) -> k b e", e=dj_cols)
        nc.sync.dma_start(out=v_v[:, :n_kb, :],
                          in_=v_r[:, :, dj * dj_cols:(dj + 1) * dj_cols])
        for qb in range(n_qb):
            mq = min(128, nq - qb * 128)
            y_ps = py.tile([128, dj_cols], F32, tag="y")
            for kb in range(n_kb):
                nc.tensor.matmul(
                    y_ps[:mq, :],
                    exp_v[:, kb, qb * 128:qb * 128 + mq],
                    v_v[:, kb, :],
                    start=(kb == 0), stop=(kb == n_kb - 1))
            y_t = yp.tile([128, dj_cols], BF16, tag="o")
            nc.vector.tensor_scalar_mul(
                y_t[:mq, :], y_ps[:mq, :], rq_t[:mq, qb:qb + 1])
            nc.sync.dma_start(
                out=y_out[qb * 128:qb * 128 + mq,
                          dj * dj_cols:(dj + 1) * dj_cols],
                in_=y_t[:mq, :])


def _attn_params(si):
    """(n, d, nq, d_pv, kbw) for scale si. Scale 0: full queries per core,
    V-column quarter (its n=128 makes query-splitting dispatch-bound);
    scales 1/2: query quarter, full V columns. kbw = keys per packed K
    load (256 gives 512B+ contiguous DMA runs)."""
    n, d = SCALES[si]
    if si == 0:
        return n, d, n, d // 4, 128
    return n, d, n // 4, d, 256


def _build_attn():
    """One launch, all 3 scales. Per core = (sample, query quarter)."""
    nc = _bacc()
    ins, outs = [], []
    for si in range(3):
        n, d, nq, d_pv, kbw = _attn_params(si)
        n_dc = d // 128
        n_g = n_dc // 32
        n_kbp = n // kbw
        ins.append((
            nc.dram_tensor(f"q{si}", [128, n_dc * nq], BF16,
                           kind="ExternalInput").ap(),
            nc.dram_tensor(f"k{si}", [n_kbp, n_g, 128, 32 * kbw], BF16,
                           kind="ExternalInput").ap(),
            nc.dram_tensor(f"v{si}", [n, d_pv], BF16,
                           kind="ExternalInput").ap(),
        ))
        outs.append(
            nc.dram_tensor(f"y{si}", [nq, d_pv], BF16,
                           kind="ExternalOutput").ap())
    with tile.TileContext(nc) as tc:
        with tc.tile_pool(name="qp", bufs=2) as qp, \
             tc.tile_pool(name="kp", bufs=3) as kp, \
             tc.tile_pool(name="ep", bufs=2) as ep, \
             tc.tile_pool(name="vp", bufs=2) as vp, \
             tc.tile_pool(name="yp", bufs=4) as yp, \
             tc.tile_pool(name="sp", bufs=2) as sp, \
             tc.tile_pool(name="pp", bufs=3, space="PSUM") as pp, \
             tc.tile_pool(name="py", bufs=3, space="PSUM") as py, \
             tc.tile_pool(name="pq", bufs=2, space="PSUM") as pq:
            pools = (qp, kp, ep, vp, yp, sp, pp, py, pq)
            # big scale first: its long QK phase overlaps later scales' DMA
            for si in (2, 1, 0):
                n, d, nq, d_pv, kbw = _attn_params(si)
                q_in, k_in, v_in = ins[si]
                _emit_attn_scale(nc, pools, q_in, k_in, v_in, outs[si],
                                 n, d, nq, d_pv, kbw)
    nc.compile()
    return nc


# ---------------------------------------------------------------- launch C
def _build_conv():
    """Per core: y2pad [2,768,66,66] bf16, woT [9,768,768] bf16, bo [768]
    -> out [2,768,4096] f32 with bias + LeakyReLU(0.2)."""
    nc = _bacc()
    x_in = nc.dram_tensor("y2pad", [FRAMES_PER_CORE, C, 66 * 66], BF16,
                          kind="ExternalInput").ap()
    w_in = nc.dram_tensor("woT", [9, C, C], BF16, kind="ExternalInput").ap()
    b_in = nc.dram_tensor("bo", [C], F32, kind="ExternalInput").ap()
    out = nc.dram_tensor("out", [FRAMES_PER_CORE, C, H * W], F32,
                         kind="ExternalOutput").ap()
    CC = C // 128
    NR = 8  # output rows per block (N = NR*64 = 512, PSUM bank limit)
    n_rb = H // NR
    with tile.TileContext(nc) as tc:
        with tc.tile_pool(name="wp", bufs=1) as wp, \
             tc.tile_pool(name="xp", bufs=2) as xp, \
             tc.tile_pool(name="op", bufs=3) as op, \
             tc.tile_pool(name="pp", bufs=3, space="PSUM") as pp:
            # all weights resident: [128(ic%128), 9, CC(ic//128), 768(oc)]
            w_t = wp.tile([128, 9, CC, C], BF16)
            nc.sync.dma_start(
                out=w_t, in_=w_in.rearrange("s (c k) o -> k s c o", k=128))
            bias_t = wp.tile([128, CC], F32)
            nc.sync.dma_start(out=bias_t,
                              in_=b_in.rearrange("(c k) -> k c", k=128))
            for f in range(FRAMES_PER_CORE):
                x_t = xp.tile([128, CC, 66 * 66], BF16)
                nc.sync.dma_start(
                    out=x_t, in_=x_in[f].rearrange("(c k) p -> k c p", k=128))
                x_v = x_t.rearrange("k c (r q) -> k c r q", r=66)
                for oc in range(CC):
                    for rb in range(n_rb):
                        ps = pp.tile([128, NR * 64], F32)
                        first = True
                        for dy in range(3):
                            for dx in range(3):
                                for ic in range(CC):
                                    y0 = rb * NR + dy
                                    rhs = x_v[:, ic, y0:y0 + NR, dx:dx + 64]
                                    nc.tensor.matmul(
                                        ps,
                                        w_t[:, dy * 3 + dx, ic,
                                            oc * 128:(oc + 1) * 128],
                                        rhs,
                                        start=first,
                                        stop=(dy == 2 and dx == 2 and ic == CC - 1))
                                    first = False
                        zt = op.tile([128, NR * 64], F32, tag="zt")
                        nc.scalar.activation(
                            out=zt, in_=ps,
                            func=mybir.ActivationFunctionType.Identity,
                            bias=bias_t[:, oc:oc + 1], scale=1.0)
                        lt = op.tile([128, NR * 64], F32, tag="lt")
                        nc.vector.tensor_scalar_mul(lt, zt, 0.2)
                        ot = op.tile([128, NR * 64], F32, tag="ot")
                        nc.vector.tensor_tensor(
                            out=ot, in0=zt, in1=lt, op=mybir.AluOpType.max)
                        nc.sync.dma_start(
                            out=out[f, oc * 128:(oc + 1) * 128,
                                    rb * (NR * 64):(rb + 1) * (NR * 64)],
                            in_=ot)
    nc.compile()
    return nc


def _build_conv_wino():
    """1D (width) Winograd F(2,3) conv: 1.5x fewer MACs than direct.
    Per core: y2pad [2,768,66,66] bf16, wWx [12,768,768] bf16 (px*3+dy,
    ic, oc = G-transformed weights), bo [768] f32 -> out [2,768,4096] f32.

    Per 16-output-row batch: T1 = B^T-combine of input cols (4 px slices,
    DVE); per (px, oc-chunk): PSUM accumulates sum_dy sum_ic W~[px,dy]^T @
    T1[rows+dy]; DVE A^T-combines the 4 px results into even/odd output
    columns; ACT applies bias + LeakyReLU."""
    nc = _bacc()
    x_in = nc.dram_tensor("y2pad", [FRAMES_PER_CORE, C, 66 * 66], BF16,
                          kind="ExternalInput").ap()
    w_in = nc.dram_tensor("wWx", [12, C, C], BF16, kind="ExternalInput").ap()
    b_in = nc.dram_tensor("bo", [C], F32, kind="ExternalInput").ap()
    out = nc.dram_tensor("out", [FRAMES_PER_CORE, C, H * W], F32,
                         kind="ExternalOutput").ap()
    CC = C // 128
    ADD, SUB = mybir.AluOpType.add, mybir.AluOpType.subtract
    with tile.TileContext(nc) as tc:
        with tc.tile_pool(name="wp", bufs=1) as wp, \
             tc.tile_pool(name="xp", bufs=1) as xp, \
             tc.tile_pool(name="tp", bufs=1) as tp, \
             tc.tile_pool(name="ap", bufs=1) as acp, \
             tc.tile_pool(name="op", bufs=2) as op, \
             tc.tile_pool(name="bp", bufs=1) as bp, \
             tc.tile_pool(name="pp", bufs=3, space="PSUM") as pp:
            bias_t = bp.tile([128, CC], F32)
            nc.sync.dma_start(out=bias_t,
                              in_=b_in.rearrange("(c k) -> k c", k=128))
            # all 12 transformed-weight matrices resident (one pass over x;
            # chunked loads so the first matmuls start after chunk 0)
            w_t = wp.tile([128, 12, CC, C], BF16)
            w_r = w_in.rearrange("s (c k) o -> k s c o", k=128)
            for s in range(12):
                nc.sync.dma_start(out=w_t[:, s], in_=w_r[:, s])
            for f in range(FRAMES_PER_CORE):
                for tb in range(4):  # 16-output-row batches
                    y0 = tb * 16
                    x_t = xp.tile([128, CC, 18 * 66], BF16, tag="x")
                    nc.sync.dma_start(
                        out=x_t,
                        in_=x_in[f][:, y0 * 66:(y0 + 18) * 66].rearrange(
                            "(c k) p -> k c p", k=128))
                    # xe[..., t, 0] = col 2t, xe[..., t, 1] = col 2t+1
                    xe = x_t.rearrange("k c (r t two) -> k c r t two",
                                       two=2, t=33)
                    t1 = tp.tile([128, CC, 18, 128], BF16, tag="t1")
                    t1v = t1.rearrange("k c r (p t) -> k c r p t", p=4)
                    # u0=d0-d2, u1=d1+d2, u2=d2-d1, u3=d1-d3
                    nc.vector.tensor_tensor(
                        out=t1v[:, :, :, 0, :], op=SUB,
                        in0=xe[:, :, :, 0:32, 0], in1=xe[:, :, :, 1:33, 0])
                    nc.vector.tensor_tensor(
                        out=t1v[:, :, :, 1, :], op=ADD,
                        in0=xe[:, :, :, 0:32, 1], in1=xe[:, :, :, 1:33, 0])
                    nc.vector.tensor_tensor(
                        out=t1v[:, :, :, 2, :], op=SUB,
                        in0=xe[:, :, :, 1:33, 0], in1=xe[:, :, :, 0:32, 1])
                    nc.vector.tensor_tensor(
                        out=t1v[:, :, :, 3, :], op=SUB,
                        in0=xe[:, :, :, 0:32, 1], in1=xe[:, :, :, 1:33, 1])
                    acc = acp.tile([128, CC, 16, 64], F32, tag="acc")
                    accv = acc.rearrange("k c r (t two) -> k c r t two",
                                         two=2)
                    for px in range(4):
                        for occ in range(CC):
                            psz = pp.tile([128, 512], F32, tag="z")
                            first = True
                            for dy in range(3):
                                for ic in range(CC):
                                    nc.tensor.matmul(
                                        psz,
                                        w_t[:, px * 3 + dy, ic,
                                            occ * 128:(occ + 1) * 128],
                                        t1v[:, ic, dy:dy + 16, px, :],
                                        start=first,
                                        stop=(dy == 2 and ic == CC - 1))
                                    first = False
                            zv = psz.rearrange("k (r t) -> k r t", r=16)
                            ev = accv[:, occ, :, :, 0]
                            od = accv[:, occ, :, :, 1]
                            # A^T: even = z0+z1+z2 ; odd = z1-z2-z3
                            if px == 0:
                                nc.vector.tensor_copy(out=ev, in_=zv)
                            elif px == 1:
                                nc.vector.tensor_tensor(
                                    out=ev, op=ADD, in0=ev, in1=zv)
                                nc.vector.tensor_copy(out=od, in_=zv)
                            elif px == 2:
                                nc.vector.tensor_tensor(
                                    out=ev, op=ADD, in0=ev, in1=zv)
                                nc.vector.tensor_tensor(
                                    out=od, op=SUB, in0=od, in1=zv)
                            else:
                                nc.vector.tensor_tensor(
                                    out=od, op=SUB, in0=od, in1=zv)
                    for occ in range(CC):
                        zt = op.tile([128, 16 * 64], F32, tag="zt")
                        nc.scalar.activation(
                            out=zt, in_=acc[:, occ],
                            func=mybir.ActivationFunctionType.Identity,
                            bias=bias_t[:, occ:occ + 1], scale=1.0)
                        lt = op.tile([128, 16 * 64], F32, tag="lt")
                        nc.vector.tensor_scalar_mul(lt, zt, 0.2)
                        ot = op.tile([128, 16 * 64], F32, tag="ot")
                        nc.vector.tensor_tensor(
                            out=ot, in0=zt, in1=lt,
                            op=mybir.AluOpType.max)
                        nc.sync.dma_start(
                            out=out[f, occ * 128:(occ + 1) * 128,
                                    tb * 1024:(tb + 1) * 1024],
                            in_=ot)
    nc.compile()
    return nc


# winograd weight transform (host, weight preprocessing)
_GX = np.array([[1.0, 0.0, 0.0],
                [0.5, 0.5, 0.5],
                [0.5, -0.5, 0.5],
                [0.0, 0.0, 1.0]], dtype=np.float32)

CONV_IMPL = "wino"  # "wino" | "direct"


# ------------------------------------------------------------------- host
def _pack_q(qsd):
    """[nq, d] bf16 -> [128, n_dc*nq] contiguous partition-major tiles."""
    nq, d = qsd.shape
    n_dc = d // 128
    return np.ascontiguousarray(
        qsd.T.reshape(n_dc, 128, nq).transpose(1, 0, 2)).reshape(
            128, n_dc * nq)


def _pack_k(ksd, kbw):
    """[n, d] bf16 -> [n_kbp, n_g, 128, 32*kbw] contiguous K tiles."""
    n, d = ksd.shape
    n_g = d // (32 * 128)
    n_kbp = n // kbw
    kt = ksd.T.reshape(n_g, 32, 128, n_kbp, kbw)
    return np.ascontiguousarray(kt.transpose(3, 0, 2, 1, 4)).reshape(
        n_kbp, n_g, 128, 32 * kbw)


def _windows(z, si, ph, pw):
    """z [bt, c, h, w] -> [b, n, D] for scale si."""
    oh, ow = H // ph, W // pw
    zz = z[:, si * DK:(si + 1) * DK].reshape(B, T, DK, oh, ph, ow, pw)
    zz = zz.transpose(0, 1, 3, 5, 2, 4, 6)
    return np.ascontiguousarray(zz.reshape(B, T * oh * ow, DK * ph * pw))


def _unwindows(y, si, ph, pw):
    """y [b, n, D] -> [bt, DK, h, w] for scale si."""
    oh, ow = H // ph, W // pw
    yy = y.reshape(B, T, oh, ow, DK, ph, pw).transpose(0, 1, 4, 2, 5, 3, 6)
    return yy.reshape(BT, DK, H, W)


def _get(name, builder, *args):
    key = (name,) + args
    if key not in _BUILD_CACHE:
        _BUILD_CACHE[key] = builder(*args)
    return _BUILD_CACHE[key]


def kernel(x, m, wq, bq, wk, bk, wv, bv, wo, bo, b, c):
    x = np.asarray(x, dtype=np.float32)
    assert x.shape == (BT, C, H, W) and int(b) == B and int(c) == C
    cores = list(range(N_CORES))

    # ---- launch A: QKV projections, 2 frames/core
    wT = np.ascontiguousarray(np.concatenate(
        [np.asarray(w)[:, :, 0, 0].T for w in (wq, wk, wv)], axis=1,
        dtype=np.float32)).astype(BF16NP)
    bqkv = np.stack([np.asarray(bq), np.asarray(bk), np.asarray(bv)]
                    ).astype(np.float32)
    x_flat = x.reshape(BT, C, H * W).astype(BF16NP)
    nc_a = _get("proj", _build_proj)
    in_maps = [{"x2": np.ascontiguousarray(
                    x_flat[i * FRAMES_PER_CORE:(i + 1) * FRAMES_PER_CORE]),
                "wT": wT, "bqkv": bqkv} for i in cores]
    res = _run(nc_a, in_maps, cores, "proj")
    qkv = np.concatenate([np.asarray(r["qkv"]) for r in res.results], axis=1)
    q_all = qkv[0].reshape(BT, C, H, W)
    k_all = qkv[1].reshape(BT, C, H, W)
    v_all = qkv[2].reshape(BT, C, H, W)

    # ---- launch B: all scales; scales 1/2 split queries, scale 0 splits
    # V columns (per sample, 4 cores each)
    nc_b = _get("attn", _build_attn)
    in_maps = [dict() for _ in cores]
    for si, (pw_, ph_) in enumerate(PATCHSIZE):
        n, d, nq, d_pv, kbw = _attn_params(si)
        qw = _windows(q_all, si, ph_, pw_)   # [b, n, D] bf16
        kw = _windows(k_all, si, ph_, pw_)
        vw = _windows(v_all, si, ph_, pw_)
        kpk = [_pack_k(kw[s], kbw) for s in range(B)]
        qpk = [_pack_q(qw[s]) for s in range(B)] if si == 0 else None
        for i in cores:
            s, qq = i // 4, i % 4
            if si == 0:
                in_maps[i][f"q{si}"] = qpk[s]
                in_maps[i][f"v{si}"] = np.ascontiguousarray(
                    vw[s][:, qq * d_pv:(qq + 1) * d_pv])
            else:
                in_maps[i][f"q{si}"] = _pack_q(qw[s, qq * nq:(qq + 1) * nq])
                in_maps[i][f"v{si}"] = vw[s]
            in_maps[i][f"k{si}"] = kpk[s]
    res = _run(nc_b, in_maps, cores, "attn")
    y_scales = []
    for si, (pw_, ph_) in enumerate(PATCHSIZE):
        n, d, nq, d_pv, kbw = _attn_params(si)
        y = np.empty((B, n, d), dtype=BF16NP)
        for i in cores:
            s, qq = i // 4, i % 4
            yi = np.asarray(res.results[i][f"y{si}"])
            if si == 0:
                y[s, :, qq * d_pv:(qq + 1) * d_pv] = yi
            else:
                y[s, qq * nq:(qq + 1) * nq] = yi
        y_scales.append(_unwindows(y, si, ph_, pw_))

    y_cat = np.concatenate(y_scales, axis=1)  # [bt, C, h, w] bf16

    # ---- launch C: 3x3 conv + bias + LeakyReLU, 2 frames/core
    y_pad = np.zeros((BT, C, 66, 66), dtype=BF16NP)
    y_pad[:, :, 1:65, 1:65] = y_cat
    y_pad = y_pad.reshape(BT, C, 66 * 66)
    bo_ = np.asarray(bo, dtype=np.float32)
    if CONV_IMPL == "wino":
        # wWx[px*3+dy][i, o] = sum_kx Gx[px,kx] wo[o,i,dy,kx]
        wWx = np.einsum('pk,oidk->pdio', _GX,
                        np.asarray(wo, dtype=np.float32),
                        optimize=True).reshape(12, C, C).astype(BF16NP)
        nc_c = _get("convw", _build_conv_wino)
        in_maps = [{"y2pad": np.ascontiguousarray(
                        y_pad[i * FRAMES_PER_CORE:(i + 1) * FRAMES_PER_CORE]),
                    "wWx": np.ascontiguousarray(wWx), "bo": bo_}
                   for i in cores]
    else:
        woT = np.ascontiguousarray(
            np.asarray(wo, dtype=np.float32).transpose(2, 3, 1, 0)
            .reshape(9, C, C)).astype(BF16NP)
        nc_c = _get("conv", _build_conv)
        in_maps = [{"y2pad": np.ascontiguousarray(
                        y_pad[i * FRAMES_PER_CORE:(i + 1) * FRAMES_PER_CORE]),
                    "woT": woT, "bo": bo_} for i in cores]
    res = _run(nc_c, in_maps, cores, "conv")
    out = np.concatenate([np.asarray(r["out"], dtype=np.float32)
                          for r in res.results], axis=0)
    return out.reshape(BT, C, H, W)
